# revision 1
# baseline (speedup 1.0000x reference)
"""MCSPN Trainium2 kernel: guidance convs + softmax gates + 4-step CSPN recurrence.

Data-parallel over batch: 8 images -> 8 NeuronCores, one image per core.
Per core:
  phase A: conv3x3 (fp32r matmuls, 18 accum MMs/row) -> bias+ReLU (ACT)
           -> conv1x1 (fp32r) -> exp (ACT) -> per-row DMA scatter into
           gate layout e_all [H=128 part, 76*256 free]
  softmax: 3 adds + reciprocal + 4 muls over [128, 19*256] strided views
  phase B: 4 recurrence steps; left/right via guarded 258-wide windows of h,
           up/down via PE shift-matmuls (sub/super-diagonal fp32r matrices)
           into PSUM; gated sums on DVE + GPSIMD.
"""
import os
import sys

sys.path.insert(0, "/opt/trn_rl_repo")

import numpy as np

B, CIN, H, W = 8, 256, 128, 256
K = 19
MID = 128
KD = 4 * K  # 76
EPS = 1e-5
T_STEPS = 4
WP = W + 2  # guarded row width (258)
RG = 8      # feats rows per DMA chunk


def _build():
    import concourse.bacc as bacc
    import concourse.mybir as mybir
    import concourse.tile as tile
    from concourse import bass

    f32 = mybir.dt.float32
    f32r = mybir.dt.float32r
    Act = mybir.ActivationFunctionType
    Alu = mybir.AluOpType

    nc = bacc.Bacc("TRN2", target_bir_lowering=False)

    feats_d = nc.dram_tensor("feats", [CIN, H, W], f32, kind="ExternalInput")
    logits_d = nc.dram_tensor("logits", [K, H, W], f32, kind="ExternalInput")
    w1t_d = nc.dram_tensor("w1t", [128, 2, 9, MID], f32, kind="ExternalInput")
    bmid_d = nc.dram_tensor("bmid", [MID, 1], f32, kind="ExternalInput")
    w2t_d = nc.dram_tensor("w2t", [MID, KD], f32, kind="ExternalInput")
    b2_d = nc.dram_tensor("b2", [KD, 1], f32, kind="ExternalInput")
    sup_d = nc.dram_tensor("sup", [128, 128], f32, kind="ExternalInput")
    sdn_d = nc.dram_tensor("sdn", [128, 128], f32, kind="ExternalInput")
    out_d = nc.dram_tensor("out", [K, H, W], f32, kind="ExternalOutput")

    with tile.TileContext(nc) as tc:
        # ---- long-lived tensors ----
        with tc.tile_pool(name="persist", bufs=1) as pp, \
             tc.tile_pool(name="hpool", bufs=1) as hp:
            e_all = pp.tile([128, KD * W], f32)           # 76 KB/part
            h_a = hp.tile([128, K * WP], f32r)            # 19.6 KB/part
            h_b = hp.tile([128, K * WP], f32r)
            w2_r = pp.tile([MID, KD], f32r)
            bmid = pp.tile([MID, 1], f32)
            b2c = pp.tile([KD, 1], f32)
            s_up = pp.tile([128, 128], f32r)
            s_dn = pp.tile([128, 128], f32r)
            z32 = pp.tile([128, 64], f32)  # zeros source for f32r guard writes

            nc.vector.memset(z32[:], 0.0)
            nc.sync.dma_start(out=bmid[:], in_=bmid_d[:])
            nc.sync.dma_start(out=b2c[:], in_=b2_d[:])
            with tc.tile_pool(name="stage", bufs=1) as stp:
                w2_f = stp.tile([MID, KD], f32)
                s_up_f = stp.tile([128, 128], f32)
                s_dn_f = stp.tile([128, 128], f32)
                nc.sync.dma_start(out=w2_f[:], in_=w2t_d[:])
                nc.vector.tensor_copy(out=w2_r[:], in_=w2_f[:])
                nc.sync.dma_start(out=s_up_f[:], in_=sup_d[:])
                nc.vector.tensor_copy(out=s_up[:], in_=s_up_f[:])
                nc.sync.dma_start(out=s_dn_f[:], in_=sdn_d[:])
                nc.vector.tensor_copy(out=s_dn[:], in_=s_dn_f[:])

            # ================= phase A: guidance =================
            with tc.tile_pool(name="w1p", bufs=1) as w1p:
                w1_f = w1p.tile([128, 2, 9, MID], f32)
                w1_r = w1p.tile([128, 2, 9, MID], f32r)
                nc.sync.dma_start(out=w1_f[:], in_=w1t_d[:])
                nc.vector.tensor_copy(out=w1_r[:], in_=w1_f[:])

                with tc.tile_pool(name="frows", bufs=3) as frp, \
                     tc.tile_pool(name="xrow", bufs=3) as xrp, \
                     tc.tile_pool(name="estrip", bufs=3) as esp, \
                     tc.tile_pool(name="psA", bufs=3, space="PSUM") as psA, \
                     tc.tile_pool(name="psG", bufs=3, space="PSUM") as psG:
                    n_groups = H // RG
                    ftiles = []  # group idx -> tile [128, 2, RG, WP]
                    for gi in range(n_groups):
                        ft = frp.tile([128, 2, RG, WP], f32r, name=f"ft{gi}",
                                      tag="ft")
                        # zero guard columns (both chunks, all rows) via
                        # rounding copy (memset can't write f32r)
                        nc.vector.tensor_copy(
                            out=ft[:, :, :, 0:WP:WP - 1],
                            in_=z32[:, 0:32].rearrange(
                                "p (a b c) -> p a b c", a=2, b=RG))
                        for c in range(2):
                            nc.sync.dma_start(
                                out=ft[:, c, :, 1:W + 1],
                                in_=feats_d[c * 128:(c + 1) * 128,
                                            gi * RG:(gi + 1) * RG, :]
                                .bitcast(f32r))
                        ftiles.append(ft)

                        # process row PAIRS whose input rows (y-1..y+2) are
                        # loaded: N=512 matmuls so LDWEIGHTS hides under the
                        # moving-operand stream.
                        if gi == 0:
                            pairs = [0, 2, 4]
                        elif gi == n_groups - 1:
                            pairs = [8 * gi - 2, 8 * gi, 8 * gi + 2,
                                     8 * gi + 4, 8 * gi + 6]
                        else:
                            pairs = [8 * gi - 2, 8 * gi, 8 * gi + 2, 8 * gi + 4]
                        for y in pairs:
                            acc = psA.tile([MID, 2, W], f32, name="acc")
                            mms = []  # (lhsT_sel, rhs_ap, out_ap)
                            # ky=1 first: always valid + full N=512, so the
                            # start=True matmul covers every PSUM element
                            for ky in (1, 0, 2):
                                for c in range(2):
                                    for kx in range(3):
                                        lw = (c, ky * 3 + kx)
                                        ys, ys2 = y + ky - 1, y + ky
                                        v0 = 0 <= ys < H
                                        v1 = 0 <= ys2 < H
                                        same = (v0 and v1
                                                and ys // RG == ys2 // RG)
                                        if same:
                                            src = ftiles[ys // RG]
                                            mms.append((lw,
                                                src[:, c, ys % RG:ys % RG + 2,
                                                    kx:kx + W],
                                                acc[:, :, :]))
                                        else:
                                            if v0:
                                                src = ftiles[ys // RG]
                                                mms.append((lw,
                                                    src[:, c, ys % RG, kx:kx + W],
                                                    acc[:, 0, :]))
                                            if v1:
                                                src = ftiles[ys2 // RG]
                                                mms.append((lw,
                                                    src[:, c, ys2 % RG, kx:kx + W],
                                                    acc[:, 1, :]))
                            for i, (lw, rhs, oap) in enumerate(mms):
                                nc.tensor.matmul(
                                    out=oap, lhsT=w1_r[:, lw[0], lw[1], :],
                                    rhs=rhs, start=(i == 0),
                                    stop=(i == len(mms) - 1))
                            # relu(x + bias) -> f32r (both rows, FD=512)
                            xr = xrp.tile([MID, 2, W], f32r, name="xr")
                            nc.scalar.activation(xr[:], acc[:], Act.Relu,
                                                 bias=bmid[:], scale=1.0)
                            accg = psG.tile([KD, 2, W], f32, name="accg")
                            nc.tensor.matmul(out=accg[:], lhsT=w2_r[:],
                                             rhs=xr[:], start=True, stop=True)
                            es = esp.tile([KD, 2, W], f32, name="es")
                            nc.scalar.activation(es[:], accg[:], Act.Exp,
                                                 bias=b2c[:], scale=1.0)
                            for r in range(2):
                                nc.sync.dma_start(
                                    out=e_all[y + r:y + r + 1, :].rearrange(
                                        "p (c w) -> p c w", c=KD),
                                    in_=es[:, r, :])

            # ================= softmax over 4 directions =================
            with tc.tile_pool(name="smx", bufs=1) as sp:
                s_all = sp.tile([128, K * W], f32)
                r_all = sp.tile([128, K * W], f32)
                ev = e_all[:].rearrange("p (k d w) -> p k d w", k=K, d=4)
                sv = s_all[:].rearrange("p (k w) -> p k w", k=K)
                nc.vector.tensor_tensor(out=sv, in0=ev[:, :, 0, :],
                                        in1=ev[:, :, 1, :], op=Alu.add)
                nc.vector.tensor_tensor(out=sv, in0=sv,
                                        in1=ev[:, :, 2, :], op=Alu.add)
                nc.vector.tensor_tensor(out=sv, in0=sv,
                                        in1=ev[:, :, 3, :], op=Alu.add)
                rv = r_all[:].rearrange("p (k w) -> p k w", k=K)
                nc.vector.reciprocal(out=r_all[:], in_=s_all[:])
                for d in range(4):
                    eng = nc.vector if d % 2 == 0 else nc.gpsimd
                    eng.tensor_tensor(out=ev[:, :, d, :], in0=ev[:, :, d, :],
                                      in1=rv, op=Alu.mult)

            # ---- load h0 = logits into guarded layout ----
            hv_a = h_a[:].rearrange("p (k w) -> p k w", k=K)
            hv_b = h_b[:].rearrange("p (k w) -> p k w", k=K)
            nc.vector.tensor_copy(
                out=hv_a[:, :, 0:WP:WP - 1],
                in_=z32[:, 0:2 * K].rearrange("p (k g) -> p k g", k=K))
            nc.vector.tensor_copy(
                out=hv_b[:, :, 0:WP:WP - 1],
                in_=z32[:, 0:2 * K].rearrange("p (k g) -> p k g", k=K))
            for k in range(K):
                nc.sync.dma_start(
                    out=h_a[:, k * WP + 1:k * WP + 1 + W],
                    in_=logits_d[k].bitcast(f32r))

            # ================= phase B: recurrence =================
            if True:
                with tc.tile_pool(name="tmp", bufs=4) as tp, \
                     tc.tile_pool(name="psS", bufs=3, space="PSUM") as psS:
                    cur, nxt = h_a, h_b
                    for t in range(T_STEPS):
                        for k in range(K):
                            base = k * WP
                            hwin = cur[:, base:base + WP]
                            up_ps = psS.tile([128, WP], f32, name="up_ps")
                            dn_ps = psS.tile([128, WP], f32, name="dn_ps")
                            nc.tensor.matmul(out=up_ps[:], lhsT=s_up[:],
                                             rhs=hwin, start=True, stop=True)
                            nc.tensor.matmul(out=dn_ps[:], lhsT=s_dn[:],
                                             rhs=hwin, start=True, stop=True)
                            gl = e_all[:, (4 * k + 0) * W:(4 * k + 1) * W]
                            gr = e_all[:, (4 * k + 1) * W:(4 * k + 2) * W]
                            gu = e_all[:, (4 * k + 2) * W:(4 * k + 3) * W]
                            gd = e_all[:, (4 * k + 3) * W:(4 * k + 4) * W]
                            left = cur[:, base:base + W].bitcast(f32)
                            right = cur[:, base + 2:base + 2 + W].bitcast(f32)
                            a = tp.tile([128, W], f32, name="a")
                            b = tp.tile([128, W], f32, name="b")
                            c2 = tp.tile([128, W], f32, name="c2")
                            d2 = tp.tile([128, W], f32, name="d2")
                            nc.vector.tensor_tensor(out=a[:], in0=gl, in1=left,
                                                    op=Alu.mult)
                            nc.gpsimd.tensor_tensor(out=b[:], in0=gr, in1=right,
                                                    op=Alu.mult)
                            nc.vector.tensor_tensor(out=c2[:], in0=gu,
                                                    in1=up_ps[:, 1:W + 1],
                                                    op=Alu.mult)
                            nc.vector.tensor_tensor(out=d2[:], in0=gd,
                                                    in1=dn_ps[:, 1:W + 1],
                                                    op=Alu.mult)
                            nc.gpsimd.tensor_tensor(out=a[:], in0=a[:], in1=b[:],
                                                    op=Alu.add)
                            nc.vector.tensor_tensor(out=c2[:], in0=c2[:],
                                                    in1=d2[:], op=Alu.add)
                            nc.vector.tensor_tensor(
                                out=nxt[:, base + 1:base + 1 + W],
                                in0=a[:], in1=c2[:], op=Alu.add)
                        cur, nxt = nxt, cur

                    for k in range(K):
                        nc.sync.dma_start(
                            out=out_d[k],
                            in_=cur[:, k * WP + 1:k * WP + 1 + W].bitcast(f32))

    nc.compile()
    return nc


_NC_CACHE = None


def kernel(feats, logits, w1, gamma, beta, mean, var, w2, b2):
    global _NC_CACHE
    from concourse.bass_utils import run_bass_kernel_spmd

    feats = np.asarray(feats, dtype=np.float32)
    logits = np.asarray(logits, dtype=np.float32)
    w1 = np.asarray(w1, dtype=np.float32)
    w2 = np.asarray(w2, dtype=np.float32)
    b2 = np.asarray(b2, dtype=np.float32)
    gamma = np.asarray(gamma, dtype=np.float32)
    beta = np.asarray(beta, dtype=np.float32)
    mean = np.asarray(mean, dtype=np.float32)
    var = np.asarray(var, dtype=np.float32)

    inv = gamma / np.sqrt(var + EPS)
    w1f = (w1 * inv[:, None, None, None]).astype(np.float32)  # [MID,CIN,3,3]
    bmid = (beta - mean * inv).astype(np.float32)[:, None]    # [MID,1]
    # [cin_in_chunk 128, chunk 2, tap 9, mid 128]
    w1t = (w1f.transpose(1, 2, 3, 0)                  # [CIN,3,3,MID]
           .reshape(2, 128, 9, MID)
           .transpose(1, 0, 2, 3)).copy()
    w2t = w2.reshape(KD, MID).T.copy()                # [MID,KD]
    b2c = b2[:, None].copy()
    s_up = np.eye(128, k=1, dtype=np.float32)         # out[m]=h[m-1]
    s_dn = np.eye(128, k=-1, dtype=np.float32)        # out[m]=h[m+1]

    if _NC_CACHE is None:
        _NC_CACHE = _build()
    nc = _NC_CACHE

    in_maps = []
    for i in range(B):
        in_maps.append({
            "feats": np.ascontiguousarray(feats[i]),
            "logits": np.ascontiguousarray(logits[i]),
            "w1t": w1t, "bmid": bmid, "w2t": w2t, "b2": b2c,
            "sup": s_up, "sdn": s_dn,
        })

    trace = bool(os.environ.get("KTRACE"))
    res = run_bass_kernel_spmd(nc, in_maps, list(range(B)), trace=trace)
    if trace and res.exec_time_ns is not None:
        print(f"HW exec time: {res.exec_time_ns} ns")
    out = np.stack([res.results[i]["out"] for i in range(B)], axis=0)
    return out.astype(np.float32)


if __name__ == "__main__":
    rng = np.random.default_rng(0)
    ins = {
        "feats": rng.standard_normal((B, CIN, H, W), dtype=np.float32),
        "logits": rng.standard_normal((B, K, H, W), dtype=np.float32),
        "w1": rng.standard_normal((MID, CIN, 3, 3), dtype=np.float32) / 48.0,
        "gamma": rng.standard_normal(MID).astype(np.float32) * 0.1 + 1.0,
        "beta": rng.standard_normal(MID).astype(np.float32) * 0.1,
        "mean": rng.standard_normal(MID).astype(np.float32) * 0.1,
        "var": rng.random(MID).astype(np.float32) + 0.5,
        "w2": rng.standard_normal((KD, MID, 1, 1)).astype(np.float32) / 11.3,
        "b2": rng.standard_normal(KD).astype(np.float32) * 0.01,
    }
    o = kernel(**ins)
    print("kernel out", o.shape, o.dtype, np.abs(o).mean())



# revision 25
# speedup vs baseline: 1.2976x; 1.2976x over previous
"""MCSPN Trainium2 kernel v2: bf16 guidance convs + softmax gates + 4-step CSPN.

Data-parallel over batch: 8 images -> 8 NeuronCores, one image per core.
Host pre-pads feats/logits (x-guard columns) and casts to bf16 so every
DMA moves large contiguous packets and DVE runs in 2x mode.

Per core:
  phase A: conv3x3 over row-QUADS (N=1024 bf16 matmuls, 18 accum MMs/quad)
           -> bias+ReLU (ACT, bf16 out) -> conv1x1 (bf16) -> exp (ACT)
           -> per-row DMA scatter into gate layout e_all [128 x 19*4*256]
  softmax: 3 adds + reciprocal + 4 muls on [128, 19, 256] bf16 views (DVE 2x)
  gate pre-shift: gup[y]=gu[y+1], gdp[y]=gd[y-1] via PE shift-matmuls (once)
  phase B: 4 steps; per step: 4 gate-muls + 1 add (bf16 2x, DVE/Pool),
           up+down shift-matmuls chain-accumulated into shared PSUM chunks,
           10 chunk adds write next-h directly.
"""
import os
import sys

sys.path.insert(0, "/opt/trn_rl_repo")

import numpy as np
import ml_dtypes

B, CIN, H, W = 8, 256, 128, 256
K = 19
MID = 128
KD = 4 * K  # 76
EPS = 1e-5
T_STEPS = 4
WP = W + 2   # guarded row width (258)
RG = 16      # feats rows per DMA group
KW = K * W   # 4864 packed gate/h width
BF = ml_dtypes.bfloat16

# psum chunking of the packed [128, K*W] plane: 4x1024 + 1x768
# (each chunk is one 4KB psum slot = 2 banks; k-aligned since 1024 = 4*W)
CHUNKS = [(j * 1024, 1024) for j in range(4)] + [(4096, 768)]


def _build(debug=False):
    import concourse.bacc as bacc
    import concourse.mybir as mybir
    import concourse.tile as tile

    f32 = mybir.dt.float32
    bf16 = mybir.dt.bfloat16
    Act = mybir.ActivationFunctionType
    Alu = mybir.AluOpType

    nc = bacc.Bacc("TRN2", target_bir_lowering=False)

    feats_d = nc.dram_tensor("feats", [CIN, H, WP], bf16, kind="ExternalInput")
    logits_d = nc.dram_tensor("logits", [K, H, WP], bf16, kind="ExternalInput")
    w1t_d = nc.dram_tensor("w1t", [128, 2, 9, MID], bf16, kind="ExternalInput")
    bmid_d = nc.dram_tensor("bmid", [MID, 1], f32, kind="ExternalInput")
    w2t_d = nc.dram_tensor("w2t", [MID, KD], bf16, kind="ExternalInput")
    b2_d = nc.dram_tensor("b2", [KD, 1], f32, kind="ExternalInput")
    sup_d = nc.dram_tensor("sup", [128, 128], bf16, kind="ExternalInput")
    sdn_d = nc.dram_tensor("sdn", [128, 128], bf16, kind="ExternalInput")
    out_d = nc.dram_tensor("out", [H, KW], f32, kind="ExternalOutput")
    if debug:
        d_eall = nc.dram_tensor("d_eall", [128, K * 4 * W], bf16,
                                kind="ExternalOutput")
        d_gup = nc.dram_tensor("d_gup", [128, KW], bf16, kind="ExternalOutput")
        d_gdp = nc.dram_tensor("d_gdp", [128, KW], bf16, kind="ExternalOutput")
        d_h1 = nc.dram_tensor("d_h1", [128, K * WP], bf16,
                              kind="ExternalOutput")
        d_ps = nc.dram_tensor("d_ps", [128, KW], f32, kind="ExternalOutput")

    with nc.allow_low_precision(reason="bf16 kernel; rel-err gate is 2e-2"), \
         tile.TileContext(nc) as tc:
        with tc.tile_pool(name="persist", bufs=1) as pp:
            e_all = pp.tile([128, 4, K, W], bf16)      # gates, d-MAJOR
            h_a = pp.tile([128, K, WP], bf16)
            h_b = pp.tile([128, K, WP], bf16)
            gup = pp.tile([128, KW], bf16)             # gu shifted: gup[y]=gu[y+1]
            gdp = pp.tile([128, KW], bf16)             # gd shifted: gdp[y]=gd[y-1]
            s_up = pp.tile([128, 128], bf16)
            s_dn = pp.tile([128, 128], bf16)
            w2 = pp.tile([MID, KD], bf16)
            bmid = pp.tile([MID, 1], f32)
            b2c = pp.tile([KD, 1], f32)

            nc.sync.dma_start(out=s_up[:], in_=sup_d[:])
            nc.sync.dma_start(out=s_dn[:], in_=sdn_d[:])
            nc.sync.dma_start(out=w2[:], in_=w2t_d[:])
            nc.sync.dma_start(out=bmid[:], in_=bmid_d[:])
            nc.sync.dma_start(out=b2c[:], in_=b2_d[:])

            # h0 = logits (host-padded guards already zero)
            for k in range(K):
                nc.sync.dma_start(out=h_a[:, k, :], in_=logits_d[k])
            # h_b guard columns zero (never written by steps)
            nc.vector.memset(h_b[:, :, 0:WP:WP - 1], 0.0)

            # ================= phase A: guidance =================
            with tc.tile_pool(name="w1p", bufs=1) as w1p, \
                 tc.tile_pool(name="frows", bufs=3) as frp, \
                 tc.tile_pool(name="xrow", bufs=3) as xrp, \
                 tc.tile_pool(name="estrip", bufs=3) as esp, \
                 tc.tile_pool(name="psA", bufs=2, space="PSUM") as psA, \
                 tc.tile_pool(name="psG", bufs=2, space="PSUM") as psG:
                w1 = w1p.tile([128, 2, 9, MID], bf16)
                nc.sync.dma_start(out=w1[:], in_=w1t_d[:])

                n_groups = H // RG
                ftiles = [None] * n_groups

                def emit_quad(y):
                    # PSUM bank limit: each matmul out <= 512 f32, so the
                    # 4-row quad accumulates as two independent 2-row halves
                    acc = psA.tile([MID, 4, W], f32, name="acc")
                    for half in (0, 1):
                        y2 = y + 2 * half
                        mms = []  # (c, tap, rhs_ap, out_ap)
                        for ky in (1, 0, 2):
                            iy0 = y2 + ky - 1
                            a0 = max(0, -iy0)
                            a1 = min(2, H - iy0)
                            runs = []
                            s = iy0 + a0
                            while s < iy0 + a1:
                                e = min(iy0 + a1, (s // RG + 1) * RG)
                                runs.append((s, e))
                                s = e
                            for c in range(2):
                                for kx in range(3):
                                    for (s, e) in runs:
                                        g = s // RG
                                        rhs = ftiles[g][:, c,
                                                        s % RG:s % RG + (e - s),
                                                        kx:kx + W]
                                        oap = acc[:, 2 * half + (s - iy0):
                                                   2 * half + (e - iy0), :]
                                        mms.append((c, ky * 3 + kx, rhs, oap))
                        for i, (c, tap, rhs, oap) in enumerate(mms):
                            nc.tensor.matmul(out=oap, lhsT=w1[:, c, tap, :],
                                             rhs=rhs, start=(i == 0),
                                             stop=(i == len(mms) - 1))
                    xr = xrp.tile([MID, 4, W], bf16, name="xr")
                    nc.scalar.activation(xr[:], acc[:], Act.Relu,
                                         bias=bmid[:], scale=1.0)
                    accg = psG.tile([KD, 4, W], f32, name="accg")
                    for half in (0, 1):
                        nc.tensor.matmul(out=accg[:, 2 * half:2 * half + 2, :],
                                         lhsT=w2[:],
                                         rhs=xr[:, 2 * half:2 * half + 2, :],
                                         start=True, stop=True)
                    es = esp.tile([KD, 4, W], bf16, name="es")
                    nc.scalar.activation(es[:], accg[:], Act.Exp,
                                         bias=b2c[:], scale=1.0)
                    # es channels are d-major (d*K+k) so the scatter only
                    # permutes free dims on the dest side (partition-dim
                    # splits in DMA APs mis-lower to slot-crossing strides)
                    for r in range(4):
                        nc.sync.dma_start(
                            out=e_all[y + r:y + r + 1, :, :, :].rearrange(
                                "p d k w -> p (d k) w"),
                            in_=es[:, r, :])

                for g in range(n_groups):
                    ft = frp.tile([128, 2, RG, WP], bf16, name=f"ft{g}", tag="ft")
                    for c in range(2):
                        nc.sync.dma_start(
                            out=ft[:, c, :, :],
                            in_=feats_d[c * 128:(c + 1) * 128,
                                        g * RG:(g + 1) * RG, :])
                    ftiles[g] = ft
                    if g == 0:
                        quads = [0, 4, 8]
                    elif g == n_groups - 1:
                        quads = [g * RG - 4, g * RG, g * RG + 4, g * RG + 8,
                                 g * RG + 12]
                    else:
                        quads = [g * RG - 4, g * RG, g * RG + 4, g * RG + 8]
                    for y in quads:
                        emit_quad(y)

            # ============ softmax + gate pre-shift ============
            with tc.tile_pool(name="work", bufs=1) as wp, \
                 tc.tile_pool(name="psS", bufs=4, space="PSUM") as psS:
                u_w = wp.tile([128, KW], bf16)
                d_w = wp.tile([128, KW], bf16)
                e1f = wp.tile([128, KW], f32)      # also holds r after softmax
                e2f = wp.tile([128, KW], f32)
                e12f = wp.tile([128, KW], f32)     # also holds s in softmax
                ps_sb = wp.tile([128, KW], f32)    # psum drain staging

                ev = [e_all[:, d, :, :] for d in range(4)]
                sv = e12f[:].rearrange("p (k w) -> p k w", k=K)
                tv = ps_sb[:].rearrange("p (k w) -> p k w", k=K)
                rv = e1f[:].rearrange("p (k w) -> p k w", k=K)
                uv = u_w[:].rearrange("p (k w) -> p k w", k=K)
                dv = d_w[:].rearrange("p (k w) -> p k w", k=K)
                nc.vector.tensor_tensor(out=sv, in0=ev[0], in1=ev[1], op=Alu.add)
                nc.gpsimd.tensor_tensor(out=tv, in0=ev[2], in1=ev[3], op=Alu.add)
                nc.vector.tensor_tensor(out=sv, in0=sv, in1=tv, op=Alu.add)
                nc.vector.reciprocal_approx_fast(out=e1f[:], in_=e12f[:])
                nc.vector.tensor_tensor(out=ev[0], in0=ev[0], in1=rv, op=Alu.mult)
                nc.gpsimd.tensor_tensor(out=ev[1], in0=ev[1], in1=rv, op=Alu.mult)
                nc.vector.tensor_tensor(out=uv, in0=ev[2], in1=rv, op=Alu.mult)
                nc.gpsimd.tensor_tensor(out=dv, in0=ev[3], in1=rv, op=Alu.mult)

                # pre-shift: gup = S_dn @ gu_norm ; gdp = S_up @ gd_norm
                for src, dst, mat in ((u_w, gup, s_dn), (d_w, gdp, s_up)):
                    for (o, n) in CHUNKS:
                        ps = psS.tile([128, 1024], f32, name="shps", tag="ps")
                        for so in range(0, n, 512):
                            sn = min(512, n - so)
                            nc.tensor.matmul(out=ps[:, so:so + sn], lhsT=mat[:],
                                             rhs=src[:, o + so:o + so + sn],
                                             start=True, stop=True)
                        nc.scalar.activation(dst[:, o:o + n], ps[:, 0:n],
                                             Act.Copy, scale=1.0)

                if debug:
                    nc.sync.dma_start(
                        out=d_eall[:],
                        in_=e_all[:].rearrange("p d k w -> p (d k w)"))
                    nc.sync.dma_start(out=d_gup[:], in_=gup[:])
                    nc.sync.dma_start(out=d_gdp[:], in_=gdp[:])

                # ================= phase B: recurrence =================
                cur, nxt = h_a, h_b
                for t in range(T_STEPS):
                    curv = cur[:, :, 1:1 + W]
                    nc.vector.tensor_tensor(out=uv, in0=gup[:].rearrange(
                        "p (k w) -> p k w", k=K), in1=curv, op=Alu.mult)
                    nc.gpsimd.tensor_tensor(out=dv, in0=gdp[:].rearrange(
                        "p (k w) -> p k w", k=K), in1=curv, op=Alu.mult)
                    ev1 = e1f[:].rearrange("p (k w) -> p k w", k=K)
                    ev2 = e2f[:].rearrange("p (k w) -> p k w", k=K)
                    nc.vector.tensor_tensor(out=ev1, in0=ev[0],
                                            in1=cur[:, :, 0:W], op=Alu.mult)
                    nc.gpsimd.tensor_tensor(out=ev2, in0=ev[1],
                                            in1=cur[:, :, 2:2 + W], op=Alu.mult)
                    nc.gpsimd.tensor_tensor(out=e12f[:], in0=e1f[:], in1=e2f[:],
                                            op=Alu.add)
                    # up+down shifts chain-accumulated per psum chunk, waves
                    # sized to keep LDWEIGHTS loaded across several matmuls;
                    # ACT drains each chunk to SBUF (Pool can't touch PSUM)
                    for wave in (CHUNKS[0:4], CHUNKS[4:5]):
                        pss = []
                        for (o, n) in wave:
                            ps = psS.tile([128, 1024], f32, name="bps", tag="ps")
                            for so in range(0, n, 512):
                                sn = min(512, n - so)
                                nc.tensor.matmul(out=ps[:, so:so + sn],
                                                 lhsT=s_up[:],
                                                 rhs=u_w[:, o + so:o + so + sn],
                                                 start=True, stop=False)
                            pss.append(ps)
                        for (o, n), ps in zip(wave, pss):
                            for so in range(0, n, 512):
                                sn = min(512, n - so)
                                nc.tensor.matmul(out=ps[:, so:so + sn],
                                                 lhsT=s_dn[:],
                                                 rhs=d_w[:, o + so:o + so + sn],
                                                 start=False, stop=True)
                        for (o, n), ps in zip(wave, pss):
                            nc.scalar.activation(ps_sb[:, o:o + n], ps[:, 0:n],
                                                 Act.Copy, scale=1.0)
                    nc.vector.tensor_tensor(
                        out=nxt[:, :, 1:1 + W],
                        in0=ps_sb[:].rearrange("p (k w) -> p k w", k=K),
                        in1=e12f[:].rearrange("p (k w) -> p k w", k=K),
                        op=Alu.add)
                    if debug and t == 0:
                        nc.sync.dma_start(
                            out=d_h1[:],
                            in_=nxt[:].rearrange("p k w -> p (k w)"))
                        nc.sync.dma_start(out=d_ps[:], in_=ps_sb[:])
                    cur, nxt = nxt, cur

                with tc.tile_pool(name="outp", bufs=1) as op_:
                    of32 = op_.tile([128, KW], f32)
                    nc.scalar.activation(
                        of32[:].rearrange("p (k w) -> p k w", k=K),
                        cur[:, :, 1:1 + W], Act.Copy, scale=1.0)
                    nc.sync.dma_start(out=out_d[:], in_=of32[:])

    nc.compile()
    return nc


_NC_CACHE = None


def kernel(feats, logits, w1, gamma, beta, mean, var, w2, b2):
    global _NC_CACHE
    from concourse.bass_utils import run_bass_kernel_spmd

    feats = np.asarray(feats, dtype=np.float32)
    logits = np.asarray(logits, dtype=np.float32)
    w1 = np.asarray(w1, dtype=np.float32)
    w2 = np.asarray(w2, dtype=np.float32)
    b2 = np.asarray(b2, dtype=np.float32)
    gamma = np.asarray(gamma, dtype=np.float32)
    beta = np.asarray(beta, dtype=np.float32)
    mean = np.asarray(mean, dtype=np.float32)
    var = np.asarray(var, dtype=np.float32)

    inv = gamma / np.sqrt(var + EPS)
    w1f = w1 * inv[:, None, None, None]               # [MID,CIN,3,3]
    bmid = (beta - mean * inv).astype(np.float32)[:, None]
    # [cin_in_chunk 128, chunk 2, tap 9, mid 128]
    w1t = np.ascontiguousarray(
        w1f.transpose(1, 2, 3, 0)                     # [CIN,3,3,MID]
        .reshape(2, 128, 9, MID)
        .transpose(1, 0, 2, 3)).astype(BF)
    # channel order d-major (c' = d*K + k) to keep the gate scatter free of
    # partition-dim splits
    w2dm = w2.reshape(K, 4, MID).transpose(1, 0, 2).reshape(KD, MID)
    w2t = np.ascontiguousarray(w2dm.T).astype(BF)
    b2c = np.ascontiguousarray(
        b2.reshape(K, 4).T.reshape(KD)[:, None]).astype(np.float32)
    s_up = np.eye(128, k=1, dtype=np.float32).astype(BF)   # out[m]=in[m-1]
    s_dn = np.eye(128, k=-1, dtype=np.float32).astype(BF)  # out[m]=in[m+1]

    feats_p = np.zeros((B, CIN, H, WP), dtype=BF)
    feats_p[:, :, :, 1:1 + W] = feats.astype(BF)
    logits_p = np.zeros((B, K, H, WP), dtype=BF)
    logits_p[:, :, :, 1:1 + W] = logits.astype(BF)

    debug = bool(os.environ.get("KDEBUG"))
    if _NC_CACHE is None:
        _NC_CACHE = _build(debug=debug)
    nc = _NC_CACHE

    in_maps = []
    for i in range(B):
        in_maps.append({
            "feats": feats_p[i],
            "logits": logits_p[i],
            "w1t": w1t, "bmid": bmid, "w2t": w2t, "b2": b2c,
            "sup": s_up, "sdn": s_dn,
        })

    trace = bool(os.environ.get("KTRACE"))
    res = run_bass_kernel_spmd(nc, in_maps, list(range(B)), trace=trace)
    if trace and res.exec_time_ns is not None:
        print(f"HW exec time: {res.exec_time_ns} ns")
    if debug:
        kernel.dbg = {k: np.asarray(res.results[0][k], dtype=np.float32)
                      for k in ("d_eall", "d_gup", "d_gdp", "d_h1", "d_ps")}
    out = np.stack([res.results[i]["out"] for i in range(B)], axis=0)
    # [B, H, K*W] -> [B, K, H, W]
    out = out.reshape(B, H, K, W).transpose(0, 2, 1, 3)
    return np.ascontiguousarray(out.astype(np.float32))


if __name__ == "__main__":
    rng = np.random.default_rng(0)
    ins = {
        "feats": rng.standard_normal((B, CIN, H, W), dtype=np.float32),
        "logits": rng.standard_normal((B, K, H, W), dtype=np.float32),
        "w1": rng.standard_normal((MID, CIN, 3, 3), dtype=np.float32) / 48.0,
        "gamma": rng.standard_normal(MID).astype(np.float32) * 0.1 + 1.0,
        "beta": rng.standard_normal(MID).astype(np.float32) * 0.1,
        "mean": rng.standard_normal(MID).astype(np.float32) * 0.1,
        "var": rng.random(MID).astype(np.float32) + 0.5,
        "w2": rng.standard_normal((KD, MID, 1, 1)).astype(np.float32) / 11.3,
        "b2": rng.standard_normal(KD).astype(np.float32) * 0.01,
    }
    o = kernel(**ins)
    print("kernel out", o.shape, o.dtype, np.abs(o).mean())


# revision 32
# speedup vs baseline: 1.5839x; 1.2206x over previous
"""MCSPN Trainium2 kernel v2: bf16 guidance convs + softmax gates + 4-step CSPN.

Data-parallel over batch: 8 images -> 8 NeuronCores, one image per core.
Host pre-pads feats/logits (x-guard columns) and casts to bf16 so every
DMA moves large contiguous packets and DVE runs in 2x mode.

Per core:
  phase A: conv3x3 over row-QUADS (N=1024 bf16 matmuls, 18 accum MMs/quad)
           -> bias+ReLU (ACT, bf16 out) -> conv1x1 (bf16) -> exp (ACT)
           -> per-row DMA scatter into gate layout e_all [128 x 19*4*256]
  softmax: 3 adds + reciprocal + 4 muls on [128, 19, 256] bf16 views (DVE 2x)
  gate pre-shift: gup[y]=gu[y+1], gdp[y]=gd[y-1] via PE shift-matmuls (once)
  phase B: 4 steps; per step: 4 gate-muls + 1 add (bf16 2x, DVE/Pool),
           up+down shift-matmuls chain-accumulated into shared PSUM chunks,
           10 chunk adds write next-h directly.
"""
import os
import sys

sys.path.insert(0, "/opt/trn_rl_repo")

import numpy as np
import ml_dtypes

B, CIN, H, W = 8, 256, 128, 256
K = 19
MID = 128
KD = 4 * K  # 76
EPS = 1e-5
T_STEPS = 4
WP = W + 2   # guarded row width (258)
RG = 16      # feats rows per DMA group
KW = K * W   # 4864 packed gate/h width
BF = ml_dtypes.bfloat16

# psum chunking of the packed [128, K*W] plane: 4x1024 + 1x768
# (each chunk is one 4KB psum slot = 2 banks; k-aligned since 1024 = 4*W)
CHUNKS = [(j * 1024, 1024) for j in range(4)] + [(4096, 768)]


def _build(debug=False):
    import concourse.bacc as bacc
    import concourse.mybir as mybir
    import concourse.tile as tile

    f32 = mybir.dt.float32
    bf16 = mybir.dt.bfloat16
    Act = mybir.ActivationFunctionType
    Alu = mybir.AluOpType

    nc = bacc.Bacc("TRN2", target_bir_lowering=False)

    feats_d = nc.dram_tensor("feats", [CIN, H, WP], bf16, kind="ExternalInput")
    logits_d = nc.dram_tensor("logits", [K, H, WP], bf16, kind="ExternalInput")
    w1t_d = nc.dram_tensor("w1t", [128, 2, 9, MID], bf16, kind="ExternalInput")
    bmid_d = nc.dram_tensor("bmid", [MID, 1], f32, kind="ExternalInput")
    w2t_d = nc.dram_tensor("w2t", [MID, KD], bf16, kind="ExternalInput")
    b2_d = nc.dram_tensor("b2", [KD, 1], f32, kind="ExternalInput")
    sup_d = nc.dram_tensor("sup", [128, 128], bf16, kind="ExternalInput")
    sdn_d = nc.dram_tensor("sdn", [128, 128], bf16, kind="ExternalInput")
    ident_d = nc.dram_tensor("ident", [128, 128], bf16, kind="ExternalInput")
    out_d = nc.dram_tensor("out", [H, KW], f32, kind="ExternalOutput")
    if debug:
        d_eall = nc.dram_tensor("d_eall", [128, K * 4 * W], bf16,
                                kind="ExternalOutput")
        d_gup = nc.dram_tensor("d_gup", [128, KW], bf16, kind="ExternalOutput")
        d_gdp = nc.dram_tensor("d_gdp", [128, KW], bf16, kind="ExternalOutput")
        d_h1 = nc.dram_tensor("d_h1", [128, K * WP], bf16,
                              kind="ExternalOutput")
        d_ps = nc.dram_tensor("d_ps", [128, KW], f32, kind="ExternalOutput")

    with nc.allow_low_precision(reason="bf16 kernel; rel-err gate is 2e-2"), \
         tile.TileContext(nc) as tc:
        with tc.tile_pool(name="persist", bufs=1) as pp:
            e_all = pp.tile([128, 4, K, W], bf16)      # gates, d-MAJOR
            h_a = pp.tile([128, K, WP], bf16)
            h_b = pp.tile([128, K, WP], bf16)
            gup = pp.tile([128, KW], bf16)             # gu shifted: gup[y]=gu[y+1]
            gdp = pp.tile([128, KW], bf16)             # gd shifted: gdp[y]=gd[y-1]
            s_up = pp.tile([128, 128], bf16)
            s_dn = pp.tile([128, 128], bf16)
            ident = pp.tile([128, 128], bf16)
            w2 = pp.tile([MID, KD], bf16)
            bmid = pp.tile([MID, 1], f32)
            b2c = pp.tile([KD, 1], f32)

            nc.sync.dma_start(out=s_up[:], in_=sup_d[:])
            nc.sync.dma_start(out=s_dn[:], in_=sdn_d[:])
            nc.sync.dma_start(out=ident[:], in_=ident_d[:])
            nc.sync.dma_start(out=w2[:], in_=w2t_d[:])
            nc.sync.dma_start(out=bmid[:], in_=bmid_d[:])
            nc.sync.dma_start(out=b2c[:], in_=b2_d[:])

            # h0 = logits (host-padded guards already zero)
            for k in range(K):
                nc.sync.dma_start(out=h_a[:, k, :], in_=logits_d[k])
            # h_b guard columns zero (never written by steps)
            nc.vector.memset(h_b[:, :, 0:WP:WP - 1], 0.0)

            # ================= phase A: guidance =================
            with tc.tile_pool(name="w1p", bufs=1) as w1p, \
                 tc.tile_pool(name="frows", bufs=3) as frp, \
                 tc.tile_pool(name="xrow", bufs=3) as xrp, \
                 tc.tile_pool(name="estrip", bufs=3) as esp, \
                 tc.tile_pool(name="psA", bufs=2, space="PSUM") as psA, \
                 tc.tile_pool(name="psG", bufs=2, space="PSUM") as psG:
                w1 = w1p.tile([128, 2, 9, MID], bf16)
                nc.sync.dma_start(out=w1[:], in_=w1t_d[:])

                n_groups = H // RG
                ftiles = [None] * n_groups

                def emit_quad(y):
                    # PSUM bank limit: each matmul out <= 512 f32, so the
                    # 4-row quad accumulates as two independent 2-row halves
                    acc = psA.tile([MID, 4, W], f32, name="acc")
                    for half in (0, 1):
                        y2 = y + 2 * half
                        mms = []  # (c, tap, rhs_ap, out_ap)
                        for ky in (1, 0, 2):
                            iy0 = y2 + ky - 1
                            a0 = max(0, -iy0)
                            a1 = min(2, H - iy0)
                            runs = []
                            s = iy0 + a0
                            while s < iy0 + a1:
                                e = min(iy0 + a1, (s // RG + 1) * RG)
                                runs.append((s, e))
                                s = e
                            for c in range(2):
                                for kx in range(3):
                                    for (s, e) in runs:
                                        g = s // RG
                                        rhs = ftiles[g][:, c,
                                                        s % RG:s % RG + (e - s),
                                                        kx:kx + W]
                                        oap = acc[:, 2 * half + (s - iy0):
                                                   2 * half + (e - iy0), :]
                                        mms.append((c, ky * 3 + kx, rhs, oap))
                        for i, (c, tap, rhs, oap) in enumerate(mms):
                            nc.tensor.matmul(out=oap, lhsT=w1[:, c, tap, :],
                                             rhs=rhs, start=(i == 0),
                                             stop=(i == len(mms) - 1))
                    xr = xrp.tile([MID, 4, W], bf16, name="xr")
                    nc.scalar.activation(xr[:], acc[:], Act.Relu,
                                         bias=bmid[:], scale=1.0)
                    accg = psG.tile([KD, 4, W], f32, name="accg")
                    for half in (0, 1):
                        nc.tensor.matmul(out=accg[:, 2 * half:2 * half + 2, :],
                                         lhsT=w2[:],
                                         rhs=xr[:, 2 * half:2 * half + 2, :],
                                         start=True, stop=True)
                    es = esp.tile([KD, 4, W], bf16, name="es")
                    nc.scalar.activation(es[:], accg[:], Act.Exp,
                                         bias=b2c[:], scale=1.0)
                    # es channels are d-major (d*K+k) so the scatter only
                    # permutes free dims on the dest side (partition-dim
                    # splits in DMA APs mis-lower to slot-crossing strides)
                    for r in range(4):
                        nc.sync.dma_start(
                            out=e_all[y + r:y + r + 1, :, :, :].rearrange(
                                "p d k w -> p (d k) w"),
                            in_=es[:, r, :])

                for g in range(n_groups):
                    ft = frp.tile([128, 2, RG, WP], bf16, name=f"ft{g}", tag="ft")
                    # 4-row slices so each group load spreads across 8 DMA
                    # engines (a single big DMA serializes on one engine and
                    # stalls the conv at group boundaries)
                    for c in range(2):
                        for q in range(0, RG, 4):
                            nc.sync.dma_start(
                                out=ft[:, c, q:q + 4, :],
                                in_=feats_d[c * 128:(c + 1) * 128,
                                            g * RG + q:g * RG + q + 4, :])
                    ftiles[g] = ft
                    if g == 0:
                        quads = [0, 4, 8]
                    elif g == n_groups - 1:
                        quads = [g * RG - 4, g * RG, g * RG + 4, g * RG + 8,
                                 g * RG + 12]
                    else:
                        quads = [g * RG - 4, g * RG, g * RG + 4, g * RG + 8]
                    for y in quads:
                        emit_quad(y)

            # ============ softmax + gate pre-shift ============
            with tc.tile_pool(name="work", bufs=1) as wp, \
                 tc.tile_pool(name="psS", bufs=4, space="PSUM") as psS:
                u_w = wp.tile([128, KW], bf16)
                d_w = wp.tile([128, KW], bf16)
                e1b = wp.tile([128, KW], bf16)
                e2b = wp.tile([128, KW], bf16)
                s_f = wp.tile([128, KW], f32)      # softmax sum, then r (f32)
                t_f = wp.tile([128, KW], f32)      # softmax partial
                r_bf = wp.tile([128, KW], bf16)    # r down-converted (ACT)

                ev = [e_all[:, d, :, :] for d in range(4)]
                sv = s_f[:].rearrange("p (k w) -> p k w", k=K)
                tv = t_f[:].rearrange("p (k w) -> p k w", k=K)
                rv = r_bf[:].rearrange("p (k w) -> p k w", k=K)
                uv = u_w[:].rearrange("p (k w) -> p k w", k=K)
                dv = d_w[:].rearrange("p (k w) -> p k w", k=K)
                # f32 sum + fast f32 reciprocal, then one ACT down-convert so
                # every gate-normalization multiply stays all-bf16 (DVE 2x;
                # f32->bf16 TT down-converts are ~5x slower on DVE)
                nc.vector.tensor_tensor(out=sv, in0=ev[0], in1=ev[1], op=Alu.add)
                nc.gpsimd.tensor_tensor(out=tv, in0=ev[2], in1=ev[3], op=Alu.add)
                nc.vector.tensor_tensor(out=sv, in0=sv, in1=tv, op=Alu.add)
                nc.vector.reciprocal_approx_fast(out=t_f[:], in_=s_f[:])
                nc.scalar.activation(r_bf[:], t_f[:], Act.Copy, scale=1.0)
                nc.vector.tensor_tensor(out=ev[0], in0=ev[0], in1=rv, op=Alu.mult)
                nc.gpsimd.tensor_tensor(out=ev[1], in0=ev[1], in1=rv, op=Alu.mult)
                nc.vector.tensor_tensor(out=uv, in0=ev[2], in1=rv, op=Alu.mult)
                nc.gpsimd.tensor_tensor(out=dv, in0=ev[3], in1=rv, op=Alu.mult)

                # pre-shift: gup = S_dn @ gu_norm ; gdp = S_up @ gd_norm
                for src, dst, mat in ((u_w, gup, s_dn), (d_w, gdp, s_up)):
                    for (o, n) in CHUNKS:
                        ps = psS.tile([128, 1024], f32, name="shps", tag="ps")
                        for so in range(0, n, 512):
                            sn = min(512, n - so)
                            nc.tensor.matmul(out=ps[:, so:so + sn], lhsT=mat[:],
                                             rhs=src[:, o + so:o + so + sn],
                                             start=True, stop=True)
                        nc.scalar.activation(dst[:, o:o + n], ps[:, 0:n],
                                             Act.Copy, scale=1.0)

                if debug:
                    nc.sync.dma_start(
                        out=d_eall[:],
                        in_=e_all[:].rearrange("p d k w -> p (d k w)"))
                    nc.sync.dma_start(out=d_gup[:], in_=gup[:])
                    nc.sync.dma_start(out=d_gdp[:], in_=gdp[:])

                # ================= phase B: recurrence =================
                # nxt = S_up@(gup.h) + S_dn@(gdp.h) + I@(gl.left) + I@(gr.right)
                # all accumulated in PSUM chunks; ACT drains straight to nxt
                # (avoids slow f32->bf16 TT down-converts on DVE/Pool)
                KSPL = 10  # dv/e2 mults: k<KSPL on DVE, rest on Pool
                ev1 = e1b[:].rearrange("p (k w) -> p k w", k=K)
                ev2 = e2b[:].rearrange("p (k w) -> p k w", k=K)
                gupv = gup[:].rearrange("p (k w) -> p k w", k=K)
                gdpv = gdp[:].rearrange("p (k w) -> p k w", k=K)
                cur, nxt = h_a, h_b
                for t in range(T_STEPS):
                    curv = cur[:, :, 1:1 + W]
                    nc.vector.tensor_tensor(out=uv, in0=gupv, in1=curv,
                                            op=Alu.mult)
                    nc.vector.tensor_tensor(out=dv[:, 0:KSPL],
                                            in0=gdpv[:, 0:KSPL],
                                            in1=curv[:, 0:KSPL],
                                            op=Alu.mult)
                    nc.gpsimd.tensor_tensor(out=dv[:, KSPL:K],
                                            in0=gdpv[:, KSPL:K],
                                            in1=curv[:, KSPL:K],
                                            op=Alu.mult)
                    nc.vector.tensor_tensor(out=ev1, in0=ev[0],
                                            in1=cur[:, :, 0:W], op=Alu.mult)
                    nc.vector.tensor_tensor(out=ev2[:, 0:KSPL],
                                            in0=ev[1][:, 0:KSPL],
                                            in1=cur[:, 0:KSPL, 2:2 + W],
                                            op=Alu.mult)
                    nc.gpsimd.tensor_tensor(out=ev2[:, KSPL:K],
                                            in0=ev[1][:, KSPL:K],
                                            in1=cur[:, KSPL:K, 2:2 + W],
                                            op=Alu.mult)
                    for wave in (CHUNKS[0:4], CHUNKS[4:5]):
                        pss = [psS.tile([128, 1024], f32, name="bps", tag="ps")
                               for _ in wave]
                        for wi, (wt, src) in enumerate(
                                ((s_up, u_w), (s_dn, d_w),
                                 (ident, e1b), (ident, e2b))):
                            for (o, n), ps in zip(wave, pss):
                                for so in range(0, n, 512):
                                    sn = min(512, n - so)
                                    nc.tensor.matmul(
                                        out=ps[:, so:so + sn], lhsT=wt[:],
                                        rhs=src[:, o + so:o + so + sn],
                                        start=(wi == 0), stop=(wi == 3))
                        for (o, n), ps in zip(wave, pss):
                            k0 = o // W
                            nk = n // W
                            nc.scalar.activation(
                                nxt[:, k0:k0 + nk, 1:1 + W],
                                ps[:, 0:n].rearrange("p (k w) -> p k w", k=nk),
                                Act.Copy, scale=1.0)
                    if debug and t == 0:
                        nc.sync.dma_start(
                            out=d_h1[:],
                            in_=nxt[:].rearrange("p k w -> p (k w)"))
                        nc.sync.dma_start(out=d_ps[:], in_=s_f[:])
                    cur, nxt = nxt, cur

                with tc.tile_pool(name="outp", bufs=1) as op_:
                    of32 = op_.tile([128, KW], f32)
                    nc.scalar.activation(
                        of32[:].rearrange("p (k w) -> p k w", k=K),
                        cur[:, :, 1:1 + W], Act.Copy, scale=1.0)
                    nc.sync.dma_start(out=out_d[:], in_=of32[:])

    nc.compile()
    return nc


_NC_CACHE = None


def kernel(feats, logits, w1, gamma, beta, mean, var, w2, b2):
    global _NC_CACHE
    from concourse.bass_utils import run_bass_kernel_spmd

    feats = np.asarray(feats, dtype=np.float32)
    logits = np.asarray(logits, dtype=np.float32)
    w1 = np.asarray(w1, dtype=np.float32)
    w2 = np.asarray(w2, dtype=np.float32)
    b2 = np.asarray(b2, dtype=np.float32)
    gamma = np.asarray(gamma, dtype=np.float32)
    beta = np.asarray(beta, dtype=np.float32)
    mean = np.asarray(mean, dtype=np.float32)
    var = np.asarray(var, dtype=np.float32)

    inv = gamma / np.sqrt(var + EPS)
    w1f = w1 * inv[:, None, None, None]               # [MID,CIN,3,3]
    bmid = (beta - mean * inv).astype(np.float32)[:, None]
    # [cin_in_chunk 128, chunk 2, tap 9, mid 128]
    w1t = np.ascontiguousarray(
        w1f.transpose(1, 2, 3, 0)                     # [CIN,3,3,MID]
        .reshape(2, 128, 9, MID)
        .transpose(1, 0, 2, 3)).astype(BF)
    # channel order d-major (c' = d*K + k) to keep the gate scatter free of
    # partition-dim splits
    w2dm = w2.reshape(K, 4, MID).transpose(1, 0, 2).reshape(KD, MID)
    w2t = np.ascontiguousarray(w2dm.T).astype(BF)
    b2c = np.ascontiguousarray(
        b2.reshape(K, 4).T.reshape(KD)[:, None]).astype(np.float32)
    s_up = np.eye(128, k=1, dtype=np.float32).astype(BF)   # out[m]=in[m-1]
    s_dn = np.eye(128, k=-1, dtype=np.float32).astype(BF)  # out[m]=in[m+1]
    ident = np.eye(128, dtype=np.float32).astype(BF)

    feats_p = np.zeros((B, CIN, H, WP), dtype=BF)
    feats_p[:, :, :, 1:1 + W] = feats.astype(BF)
    logits_p = np.zeros((B, K, H, WP), dtype=BF)
    logits_p[:, :, :, 1:1 + W] = logits.astype(BF)

    debug = bool(os.environ.get("KDEBUG"))
    if _NC_CACHE is None:
        _NC_CACHE = _build(debug=debug)
    nc = _NC_CACHE

    in_maps = []
    for i in range(B):
        in_maps.append({
            "feats": feats_p[i],
            "logits": logits_p[i],
            "w1t": w1t, "bmid": bmid, "w2t": w2t, "b2": b2c,
            "sup": s_up, "sdn": s_dn, "ident": ident,
        })

    trace = bool(os.environ.get("KTRACE"))
    res = run_bass_kernel_spmd(nc, in_maps, list(range(B)), trace=trace)
    if trace and res.exec_time_ns is not None:
        print(f"HW exec time: {res.exec_time_ns} ns")
    if debug:
        kernel.dbg = {k: np.asarray(res.results[0][k], dtype=np.float32)
                      for k in ("d_eall", "d_gup", "d_gdp", "d_h1", "d_ps")}
    out = np.stack([res.results[i]["out"] for i in range(B)], axis=0)
    # [B, H, K*W] -> [B, K, H, W]
    out = out.reshape(B, H, K, W).transpose(0, 2, 1, 3)
    return np.ascontiguousarray(out.astype(np.float32))


if __name__ == "__main__":
    rng = np.random.default_rng(0)
    ins = {
        "feats": rng.standard_normal((B, CIN, H, W), dtype=np.float32),
        "logits": rng.standard_normal((B, K, H, W), dtype=np.float32),
        "w1": rng.standard_normal((MID, CIN, 3, 3), dtype=np.float32) / 48.0,
        "gamma": rng.standard_normal(MID).astype(np.float32) * 0.1 + 1.0,
        "beta": rng.standard_normal(MID).astype(np.float32) * 0.1,
        "mean": rng.standard_normal(MID).astype(np.float32) * 0.1,
        "var": rng.random(MID).astype(np.float32) + 0.5,
        "w2": rng.standard_normal((KD, MID, 1, 1)).astype(np.float32) / 11.3,
        "b2": rng.standard_normal(KD).astype(np.float32) * 0.01,
    }
    o = kernel(**ins)
    print("kernel out", o.shape, o.dtype, np.abs(o).mean())


# revision 39
# speedup vs baseline: 1.8987x; 1.1988x over previous
"""MCSPN Trainium2 kernel v2: bf16 guidance convs + softmax gates + 4-step CSPN.

Data-parallel over batch: 8 images -> 8 NeuronCores, one image per core.
Host pre-pads feats/logits (x-guard columns) and casts to bf16 so every
DMA moves large contiguous packets and DVE runs in 2x mode.

Per core:
  phase A: conv3x3 over row-QUADS (N=1024 bf16 matmuls, 18 accum MMs/quad)
           -> bias+ReLU (ACT, bf16 out) -> conv1x1 (bf16) -> exp (ACT)
           -> per-row DMA scatter into gate layout e_all [128 x 19*4*256]
  softmax: 3 adds + reciprocal + 4 muls on [128, 19, 256] bf16 views (DVE 2x)
  gate pre-shift: gup[y]=gu[y+1], gdp[y]=gd[y-1] via PE shift-matmuls (once)
  phase B: 4 steps; per step: 4 gate-muls + 1 add (bf16 2x, DVE/Pool),
           up+down shift-matmuls chain-accumulated into shared PSUM chunks,
           10 chunk adds write next-h directly.
"""
import os
import sys

sys.path.insert(0, "/opt/trn_rl_repo")

import numpy as np
import ml_dtypes

B, CIN, H, W = 8, 256, 128, 256
K = 19
MID = 128
KD = 4 * K  # 76
EPS = 1e-5
T_STEPS = 4
WP = W + 2   # guarded row width (258)
RG = 16      # feats rows per DMA group
KW = K * W   # 4864 packed gate/h width
BF = ml_dtypes.bfloat16

# psum chunking of the packed [128, K*W] plane: 4x1024 + 1x768
# (each chunk is one 4KB psum slot = 2 banks; k-aligned since 1024 = 4*W)
CHUNKS = [(j * 1024, 1024) for j in range(4)] + [(4096, 768)]


def _build(debug=False):
    import concourse.bacc as bacc
    import concourse.mybir as mybir
    import concourse.tile as tile

    f32 = mybir.dt.float32
    bf16 = mybir.dt.bfloat16
    Act = mybir.ActivationFunctionType
    Alu = mybir.AluOpType

    nc = bacc.Bacc("TRN2", target_bir_lowering=False)

    feats_d = nc.dram_tensor("feats", [CIN, H, WP], bf16, kind="ExternalInput")
    logits_d = nc.dram_tensor("logits", [K, H, WP], bf16, kind="ExternalInput")
    w1t_d = nc.dram_tensor("w1t", [128, 2, 9, MID], bf16, kind="ExternalInput")
    bmid_d = nc.dram_tensor("bmid", [MID, 1], f32, kind="ExternalInput")
    w2t_d = nc.dram_tensor("w2t", [MID, KD], bf16, kind="ExternalInput")
    b2_d = nc.dram_tensor("b2", [KD, 1], f32, kind="ExternalInput")
    sup_d = nc.dram_tensor("sup", [128, 128], bf16, kind="ExternalInput")
    sdn_d = nc.dram_tensor("sdn", [128, 128], bf16, kind="ExternalInput")
    ident_d = nc.dram_tensor("ident", [128, 128], bf16, kind="ExternalInput")
    ksum_d = nc.dram_tensor("ksum", [KD, KD], bf16, kind="ExternalInput")
    out_d = nc.dram_tensor("out", [H, KW], f32, kind="ExternalOutput")
    if debug:
        d_eall = nc.dram_tensor("d_eall", [128, K * 4 * W], bf16,
                                kind="ExternalOutput")
        d_gup = nc.dram_tensor("d_gup", [128, KW], bf16, kind="ExternalOutput")
        d_gdp = nc.dram_tensor("d_gdp", [128, KW], bf16, kind="ExternalOutput")
        d_h1 = nc.dram_tensor("d_h1", [128, K * WP], bf16,
                              kind="ExternalOutput")
        d_ps = nc.dram_tensor("d_ps", [128, KW], bf16, kind="ExternalOutput")

    with nc.allow_low_precision(reason="bf16 kernel; rel-err gate is 2e-2"), \
         tile.TileContext(nc) as tc:
        with tc.tile_pool(name="persist", bufs=1) as pp:
            e_all = pp.tile([128, 4, K, W], bf16)      # gates, d-MAJOR
            h_a = pp.tile([128, K, WP], bf16)
            h_b = pp.tile([128, K, WP], bf16)
            gup = pp.tile([128, KW], bf16)             # gu shifted: gup[y]=gu[y+1]
            gdp = pp.tile([128, KW], bf16)             # gd shifted: gdp[y]=gd[y-1]
            s_up = pp.tile([128, 128], bf16)
            s_dn = pp.tile([128, 128], bf16)
            ident = pp.tile([128, 128], bf16)
            ksum = pp.tile([KD, KD], bf16)
            w2 = pp.tile([MID, KD], bf16)
            bmid = pp.tile([MID, 1], f32)
            b2c = pp.tile([KD, 1], f32)

            nc.sync.dma_start(out=s_up[:], in_=sup_d[:])
            nc.sync.dma_start(out=s_dn[:], in_=sdn_d[:])
            nc.sync.dma_start(out=ident[:], in_=ident_d[:])
            nc.sync.dma_start(out=ksum[:], in_=ksum_d[:])
            nc.sync.dma_start(out=w2[:], in_=w2t_d[:])
            nc.sync.dma_start(out=bmid[:], in_=bmid_d[:])
            nc.sync.dma_start(out=b2c[:], in_=b2_d[:])

            # h0 = logits (host-padded guards already zero)
            for k in range(K):
                nc.sync.dma_start(out=h_a[:, k, :], in_=logits_d[k])
            # h_b guard columns zero (never written by steps)
            nc.vector.memset(h_b[:, :, 0:WP:WP - 1], 0.0)

            # ================= phase A: guidance =================
            with tc.tile_pool(name="w1p", bufs=1) as w1p, \
                 tc.tile_pool(name="frows", bufs=3) as frp, \
                 tc.tile_pool(name="xrow", bufs=3) as xrp, \
                 tc.tile_pool(name="estrip", bufs=3) as esp, \
                 tc.tile_pool(name="rqp", bufs=2) as rqp, \
                 tc.tile_pool(name="psA", bufs=2, space="PSUM") as psA, \
                 tc.tile_pool(name="psG", bufs=1, space="PSUM") as psG, \
                 tc.tile_pool(name="psN", bufs=1, space="PSUM") as psN:
                w1 = w1p.tile([128, 2, 9, MID], bf16)
                nc.sync.dma_start(out=w1[:], in_=w1t_d[:])

                n_groups = H // RG
                ftiles = [None] * n_groups

                def emit_quad(y):
                    # PSUM bank limit: each matmul out <= 512 f32, so the
                    # 4-row quad accumulates as two independent 2-row halves
                    acc = psA.tile([MID, 4, W], f32, name="acc")
                    for half in (0, 1):
                        y2 = y + 2 * half
                        mms = []  # (c, tap, rhs_ap, out_ap)
                        for ky in (1, 0, 2):
                            iy0 = y2 + ky - 1
                            a0 = max(0, -iy0)
                            a1 = min(2, H - iy0)
                            runs = []
                            s = iy0 + a0
                            while s < iy0 + a1:
                                e = min(iy0 + a1, (s // RG + 1) * RG)
                                runs.append((s, e))
                                s = e
                            for c in range(2):
                                for kx in range(3):
                                    for (s, e) in runs:
                                        g = s // RG
                                        rhs = ftiles[g][:, c,
                                                        s % RG:s % RG + (e - s),
                                                        kx:kx + W]
                                        oap = acc[:, 2 * half + (s - iy0):
                                                   2 * half + (e - iy0), :]
                                        mms.append((c, ky * 3 + kx, rhs, oap))
                        for i, (c, tap, rhs, oap) in enumerate(mms):
                            nc.tensor.matmul(out=oap, lhsT=w1[:, c, tap, :],
                                             rhs=rhs, start=(i == 0),
                                             stop=(i == len(mms) - 1))
                    xr = xrp.tile([MID, 4, W], bf16, name="xr")
                    nc.scalar.activation(xr[:], acc[:], Act.Relu,
                                         bias=bmid[:], scale=1.0)
                    accg = psG.tile([KD, 4, W], f32, name="accg")
                    for half in (0, 1):
                        nc.tensor.matmul(out=accg[:, 2 * half:2 * half + 2, :],
                                         lhsT=w2[:],
                                         rhs=xr[:, 2 * half:2 * half + 2, :],
                                         start=True, stop=True)
                    es = esp.tile([KD, 4, W], bf16, name="es")
                    nc.scalar.activation(es[:], accg[:], Act.Exp,
                                         bias=b2c[:], scale=1.0)
                    # in-phase softmax: PE reduces the 4 directions per k
                    # (ksum block matrix, f32 psum), DVE fast-reciprocal +
                    # in-place normalize -- all hidden under the conv
                    sps = psN.tile([KD, 4, W], f32, name="sps")
                    for half in (0, 1):
                        nc.tensor.matmul(out=sps[:, 2 * half:2 * half + 2, :],
                                         lhsT=ksum[:],
                                         rhs=es[:, 2 * half:2 * half + 2, :],
                                         start=True, stop=True)
                    rq = rqp.tile([KD, 4, W], f32, name="rq")
                    nc.vector.reciprocal_approx_fast(out=rq[:], in_=sps[:])
                    nc.vector.tensor_tensor(out=es[:], in0=es[:], in1=rq[:],
                                            op=Alu.mult)
                    # es channels are d-major (d*K+k) so the scatter only
                    # permutes free dims on the dest side (partition-dim
                    # splits in DMA APs mis-lower to slot-crossing strides)
                    for r in range(4):
                        eng = nc.sync if r % 2 == 0 else nc.scalar
                        eng.dma_start(
                            out=e_all[y + r:y + r + 1, :, :, :].rearrange(
                                "p d k w -> p (d k) w"),
                            in_=es[:, r, :])

                for g in range(n_groups):
                    ft = frp.tile([128, 2, RG, WP], bf16, name=f"ft{g}", tag="ft")
                    # 4-row slices so each group load spreads across 8 DMA
                    # engines (a single big DMA serializes on one engine and
                    # stalls the conv at group boundaries)
                    for c in range(2):
                        for q in range(0, RG, 4):
                            nc.sync.dma_start(
                                out=ft[:, c, q:q + 4, :],
                                in_=feats_d[c * 128:(c + 1) * 128,
                                            g * RG + q:g * RG + q + 4, :])
                    ftiles[g] = ft
                    if g == 0:
                        quads = [0, 4, 8]
                    elif g == n_groups - 1:
                        quads = [g * RG - 4, g * RG, g * RG + 4, g * RG + 8,
                                 g * RG + 12]
                    else:
                        quads = [g * RG - 4, g * RG, g * RG + 4, g * RG + 8]
                    for y in quads:
                        emit_quad(y)

            # ============ gate pre-shift ============
            with tc.tile_pool(name="work", bufs=1) as wp, \
                 tc.tile_pool(name="psS", bufs=4, space="PSUM") as psS:
                u_w = wp.tile([128, KW], bf16)
                d_w = wp.tile([128, KW], bf16)
                e1b = wp.tile([128, KW], bf16)
                e2b = wp.tile([128, KW], bf16)

                ev = [e_all[:, d, :, :] for d in range(4)]
                uv = u_w[:].rearrange("p (k w) -> p k w", k=K)
                dv = d_w[:].rearrange("p (k w) -> p k w", k=K)

                # pre-shift: gup = S_dn @ gu ; gdp = S_up @ gd (normalized)
                for dsel, dst, mat in ((2, gup, s_dn), (3, gdp, s_up)):
                    for (o, n) in CHUNKS:
                        ps = psS.tile([128, 1024], f32, name="shps", tag="ps")
                        for so in range(0, n, 512):
                            sn = min(512, n - so)
                            k0 = (o + so) // W
                            nc.tensor.matmul(
                                out=ps[:, so:so + sn], lhsT=mat[:],
                                rhs=e_all[:, dsel, k0:k0 + sn // W, :],
                                start=True, stop=True)
                        nc.scalar.activation(dst[:, o:o + n], ps[:, 0:n],
                                             Act.Copy, scale=1.0)

                if debug:
                    nc.sync.dma_start(
                        out=d_eall[:],
                        in_=e_all[:].rearrange("p d k w -> p (d k w)"))
                    nc.sync.dma_start(out=d_gup[:], in_=gup[:])
                    nc.sync.dma_start(out=d_gdp[:], in_=gdp[:])

                # ================= phase B: recurrence =================
                # nxt = S_up@(gup.h) + S_dn@(gdp.h) + I@(gl.left) + I@(gr.right)
                # all accumulated in PSUM chunks; ACT drains straight to nxt
                # (avoids slow f32->bf16 TT down-converts on DVE/Pool)
                ev1 = e1b[:].rearrange("p (k w) -> p k w", k=K)
                ev2 = e2b[:].rearrange("p (k w) -> p k w", k=K)
                gupv = gup[:].rearrange("p (k w) -> p k w", k=K)
                gdpv = gdp[:].rearrange("p (k w) -> p k w", k=K)
                cur, nxt = h_a, h_b
                for t in range(T_STEPS):
                    # all four gate-mults on DVE (all-bf16 2x; Pool's software
                    # TT is ~3.5x slower and adds SBUF contention)
                    curv = cur[:, :, 1:1 + W]
                    nc.vector.tensor_tensor(out=uv, in0=gupv, in1=curv,
                                            op=Alu.mult)
                    nc.vector.tensor_tensor(out=dv, in0=gdpv, in1=curv,
                                            op=Alu.mult)
                    nc.vector.tensor_tensor(out=ev1, in0=ev[0],
                                            in1=cur[:, :, 0:W], op=Alu.mult)
                    nc.vector.tensor_tensor(out=ev2, in0=ev[1],
                                            in1=cur[:, :, 2:2 + W], op=Alu.mult)
                    for wave in (CHUNKS[0:4], CHUNKS[4:5]):
                        pss = [psS.tile([128, 1024], f32, name="bps", tag="ps")
                               for _ in wave]
                        for wi, (wt, src) in enumerate(
                                ((s_up, u_w), (s_dn, d_w),
                                 (ident, e1b), (ident, e2b))):
                            for (o, n), ps in zip(wave, pss):
                                for so in range(0, n, 512):
                                    sn = min(512, n - so)
                                    nc.tensor.matmul(
                                        out=ps[:, so:so + sn], lhsT=wt[:],
                                        rhs=src[:, o + so:o + so + sn],
                                        start=(wi == 0), stop=(wi == 3))
                        for (o, n), ps in zip(wave, pss):
                            k0 = o // W
                            nk = n // W
                            nc.scalar.activation(
                                nxt[:, k0:k0 + nk, 1:1 + W],
                                ps[:, 0:n].rearrange("p (k w) -> p k w", k=nk),
                                Act.Copy, scale=1.0)
                    if debug and t == 0:
                        nc.sync.dma_start(
                            out=d_h1[:],
                            in_=nxt[:].rearrange("p k w -> p (k w)"))
                        nc.sync.dma_start(out=d_ps[:], in_=u_w[:])
                    cur, nxt = nxt, cur

                with tc.tile_pool(name="outp", bufs=1) as op_:
                    of32 = op_.tile([128, KW], f32)
                    nc.scalar.activation(
                        of32[:].rearrange("p (k w) -> p k w", k=K),
                        cur[:, :, 1:1 + W], Act.Copy, scale=1.0)
                    nc.sync.dma_start(out=out_d[:], in_=of32[:])

    nc.compile()
    return nc


_NC_CACHE = None


def kernel(feats, logits, w1, gamma, beta, mean, var, w2, b2):
    global _NC_CACHE
    from concourse.bass_utils import run_bass_kernel_spmd

    feats = np.asarray(feats, dtype=np.float32)
    logits = np.asarray(logits, dtype=np.float32)
    w1 = np.asarray(w1, dtype=np.float32)
    w2 = np.asarray(w2, dtype=np.float32)
    b2 = np.asarray(b2, dtype=np.float32)
    gamma = np.asarray(gamma, dtype=np.float32)
    beta = np.asarray(beta, dtype=np.float32)
    mean = np.asarray(mean, dtype=np.float32)
    var = np.asarray(var, dtype=np.float32)

    inv = gamma / np.sqrt(var + EPS)
    w1f = w1 * inv[:, None, None, None]               # [MID,CIN,3,3]
    bmid = (beta - mean * inv).astype(np.float32)[:, None]
    # [cin_in_chunk 128, chunk 2, tap 9, mid 128]
    w1t = np.ascontiguousarray(
        w1f.transpose(1, 2, 3, 0)                     # [CIN,3,3,MID]
        .reshape(2, 128, 9, MID)
        .transpose(1, 0, 2, 3)).astype(BF)
    # channel order d-major (c' = d*K + k) to keep the gate scatter free of
    # partition-dim splits
    w2dm = w2.reshape(K, 4, MID).transpose(1, 0, 2).reshape(KD, MID)
    w2t = np.ascontiguousarray(w2dm.T).astype(BF)
    b2c = np.ascontiguousarray(
        b2.reshape(K, 4).T.reshape(KD)[:, None]).astype(np.float32)
    s_up = np.eye(128, k=1, dtype=np.float32).astype(BF)   # out[m]=in[m-1]
    s_dn = np.eye(128, k=-1, dtype=np.float32).astype(BF)  # out[m]=in[m+1]
    ident = np.eye(128, dtype=np.float32).astype(BF)
    # ksum[c', c] = 1 iff same k (channels d-major: k = c % 19)
    cc = np.arange(KD)
    ksum = (cc[:, None] % K == cc[None, :] % K).astype(np.float32).astype(BF)

    feats_p = np.zeros((B, CIN, H, WP), dtype=BF)
    feats_p[:, :, :, 1:1 + W] = feats.astype(BF)
    logits_p = np.zeros((B, K, H, WP), dtype=BF)
    logits_p[:, :, :, 1:1 + W] = logits.astype(BF)

    debug = bool(os.environ.get("KDEBUG"))
    if _NC_CACHE is None:
        _NC_CACHE = _build(debug=debug)
    nc = _NC_CACHE

    in_maps = []
    for i in range(B):
        in_maps.append({
            "feats": feats_p[i],
            "logits": logits_p[i],
            "w1t": w1t, "bmid": bmid, "w2t": w2t, "b2": b2c,
            "sup": s_up, "sdn": s_dn, "ident": ident, "ksum": ksum,
        })

    trace = bool(os.environ.get("KTRACE"))
    res = run_bass_kernel_spmd(nc, in_maps, list(range(B)), trace=trace)
    if trace and res.exec_time_ns is not None:
        print(f"HW exec time: {res.exec_time_ns} ns")
    if debug:
        kernel.dbg = {k: np.asarray(res.results[0][k], dtype=np.float32)
                      for k in ("d_eall", "d_gup", "d_gdp", "d_h1", "d_ps")}
    out = np.stack([res.results[i]["out"] for i in range(B)], axis=0)
    # [B, H, K*W] -> [B, K, H, W]
    out = out.reshape(B, H, K, W).transpose(0, 2, 1, 3)
    return np.ascontiguousarray(out.astype(np.float32))


if __name__ == "__main__":
    rng = np.random.default_rng(0)
    ins = {
        "feats": rng.standard_normal((B, CIN, H, W), dtype=np.float32),
        "logits": rng.standard_normal((B, K, H, W), dtype=np.float32),
        "w1": rng.standard_normal((MID, CIN, 3, 3), dtype=np.float32) / 48.0,
        "gamma": rng.standard_normal(MID).astype(np.float32) * 0.1 + 1.0,
        "beta": rng.standard_normal(MID).astype(np.float32) * 0.1,
        "mean": rng.standard_normal(MID).astype(np.float32) * 0.1,
        "var": rng.random(MID).astype(np.float32) + 0.5,
        "w2": rng.standard_normal((KD, MID, 1, 1)).astype(np.float32) / 11.3,
        "b2": rng.standard_normal(KD).astype(np.float32) * 0.01,
    }
    o = kernel(**ins)
    print("kernel out", o.shape, o.dtype, np.abs(o).mean())


# revision 41
# speedup vs baseline: 2.0170x; 1.0623x over previous
"""MCSPN Trainium2 kernel v2: bf16 guidance convs + softmax gates + 4-step CSPN.

Data-parallel over batch: 8 images -> 8 NeuronCores, one image per core.
Host pre-pads feats/logits (x-guard columns) and casts to bf16 so every
DMA moves large contiguous packets and DVE runs in 2x mode.

Per core:
  phase A: conv3x3 over row-QUADS (N=1024 bf16 matmuls, 18 accum MMs/quad)
           -> bias+ReLU (ACT, bf16 out) -> conv1x1 (bf16) -> exp (ACT)
           -> per-row DMA scatter into gate layout e_all [128 x 19*4*256]
  softmax: 3 adds + reciprocal + 4 muls on [128, 19, 256] bf16 views (DVE 2x)
  gate pre-shift: gup[y]=gu[y+1], gdp[y]=gd[y-1] via PE shift-matmuls (once)
  phase B: 4 steps; per step: 4 gate-muls + 1 add (bf16 2x, DVE/Pool),
           up+down shift-matmuls chain-accumulated into shared PSUM chunks,
           10 chunk adds write next-h directly.
"""
import os
import sys

sys.path.insert(0, "/opt/trn_rl_repo")

import numpy as np
import ml_dtypes

B, CIN, H, W = 8, 256, 128, 256
K = 19
MID = 128
KD = 4 * K  # 76
EPS = 1e-5
T_STEPS = 4
WP = W + 2   # guarded row width (258)
RG = 16      # feats rows per DMA group
KW = K * W   # 4864 packed gate/h width
BF = ml_dtypes.bfloat16

# psum chunking of the packed [128, K*W] plane: 4x1024 + 1x768
# (each chunk is one 4KB psum slot = 2 banks; k-aligned since 1024 = 4*W)
CHUNKS = [(j * 1024, 1024) for j in range(4)] + [(4096, 768)]


def _build(debug=False):
    import concourse.bacc as bacc
    import concourse.mybir as mybir
    import concourse.tile as tile

    f32 = mybir.dt.float32
    bf16 = mybir.dt.bfloat16
    Act = mybir.ActivationFunctionType
    Alu = mybir.AluOpType

    nc = bacc.Bacc("TRN2", target_bir_lowering=False)

    feats_d = nc.dram_tensor("feats", [CIN, H, WP], bf16, kind="ExternalInput")
    logits_d = nc.dram_tensor("logits", [K, H, WP], bf16, kind="ExternalInput")
    w1t_d = nc.dram_tensor("w1t", [128, 2, 9, MID], bf16, kind="ExternalInput")
    bmid_d = nc.dram_tensor("bmid", [MID, 1], f32, kind="ExternalInput")
    w2t_d = nc.dram_tensor("w2t", [MID, KD], bf16, kind="ExternalInput")
    b2_d = nc.dram_tensor("b2", [KD, 1], f32, kind="ExternalInput")
    sup_d = nc.dram_tensor("sup", [128, 128], bf16, kind="ExternalInput")
    sdn_d = nc.dram_tensor("sdn", [128, 128], bf16, kind="ExternalInput")
    ident_d = nc.dram_tensor("ident", [128, 128], bf16, kind="ExternalInput")
    ksum_d = nc.dram_tensor("ksum", [KD, KD], bf16, kind="ExternalInput")
    out_d = nc.dram_tensor("out", [H, KW], f32, kind="ExternalOutput")
    if debug:
        d_eall = nc.dram_tensor("d_eall", [128, K * 4 * W], bf16,
                                kind="ExternalOutput")
        d_gup = nc.dram_tensor("d_gup", [128, KW], bf16, kind="ExternalOutput")
        d_gdp = nc.dram_tensor("d_gdp", [128, KW], bf16, kind="ExternalOutput")
        d_h1 = nc.dram_tensor("d_h1", [128, K * WP], bf16,
                              kind="ExternalOutput")
        d_ps = nc.dram_tensor("d_ps", [128, KW], bf16, kind="ExternalOutput")

    with nc.allow_low_precision(reason="bf16 kernel; rel-err gate is 2e-2"), \
         tile.TileContext(nc) as tc:
        with tc.tile_pool(name="persist", bufs=1) as pp:
            e_all = pp.tile([128, 4, K, W], bf16)      # gates, d-MAJOR
            h_a = pp.tile([128, K, WP], bf16)
            h_b = pp.tile([128, K, WP], bf16)
            gup = pp.tile([128, KW], bf16)             # gu shifted: gup[y]=gu[y+1]
            gdp = pp.tile([128, KW], bf16)             # gd shifted: gdp[y]=gd[y-1]
            s_up = pp.tile([128, 128], bf16)
            s_dn = pp.tile([128, 128], bf16)
            ident = pp.tile([128, 128], bf16)
            ksum = pp.tile([KD, KD], bf16)
            w2 = pp.tile([MID, KD], bf16)
            bmid = pp.tile([MID, 1], f32)
            b2c = pp.tile([KD, 1], f32)

            nc.sync.dma_start(out=s_up[:], in_=sup_d[:])
            nc.sync.dma_start(out=s_dn[:], in_=sdn_d[:])
            nc.sync.dma_start(out=ident[:], in_=ident_d[:])
            nc.sync.dma_start(out=ksum[:], in_=ksum_d[:])
            nc.sync.dma_start(out=w2[:], in_=w2t_d[:])
            nc.sync.dma_start(out=bmid[:], in_=bmid_d[:])
            nc.sync.dma_start(out=b2c[:], in_=b2_d[:])

            # h0 = logits (host-padded guards already zero)
            for k in range(K):
                nc.sync.dma_start(out=h_a[:, k, :], in_=logits_d[k])
            # h_b guard columns zero (never written by steps)
            nc.vector.memset(h_b[:, :, 0:WP:WP - 1], 0.0)

            # ================= phase A: guidance =================
            with tc.tile_pool(name="w1p", bufs=1) as w1p, \
                 tc.tile_pool(name="frows", bufs=4) as frp, \
                 tc.tile_pool(name="xrow", bufs=3) as xrp, \
                 tc.tile_pool(name="estrip", bufs=3) as esp, \
                 tc.tile_pool(name="rqp", bufs=2) as rqp, \
                 tc.tile_pool(name="psA", bufs=2, space="PSUM") as psA, \
                 tc.tile_pool(name="psG", bufs=1, space="PSUM") as psG, \
                 tc.tile_pool(name="psN", bufs=1, space="PSUM") as psN:
                w1 = w1p.tile([128, 2, 9, MID], bf16)
                nc.sync.dma_start(out=w1[:], in_=w1t_d[:])

                n_groups = H // RG
                ftiles = [None] * n_groups

                def emit_quad(y):
                    # PSUM bank limit: each matmul out <= 512 f32, so the
                    # 4-row quad accumulates as two independent 2-row halves
                    acc = psA.tile([MID, 4, W], f32, name="acc")
                    for half in (0, 1):
                        y2 = y + 2 * half
                        mms = []  # (c, tap, rhs_ap, out_ap)
                        for ky in (1, 0, 2):
                            iy0 = y2 + ky - 1
                            a0 = max(0, -iy0)
                            a1 = min(2, H - iy0)
                            runs = []
                            s = iy0 + a0
                            while s < iy0 + a1:
                                e = min(iy0 + a1, (s // RG + 1) * RG)
                                runs.append((s, e))
                                s = e
                            for c in range(2):
                                for kx in range(3):
                                    for (s, e) in runs:
                                        g = s // RG
                                        rhs = ftiles[g][:, c,
                                                        s % RG:s % RG + (e - s),
                                                        kx:kx + W]
                                        oap = acc[:, 2 * half + (s - iy0):
                                                   2 * half + (e - iy0), :]
                                        mms.append((c, ky * 3 + kx, rhs, oap))
                        for i, (c, tap, rhs, oap) in enumerate(mms):
                            nc.tensor.matmul(out=oap, lhsT=w1[:, c, tap, :],
                                             rhs=rhs, start=(i == 0),
                                             stop=(i == len(mms) - 1))
                    xr = xrp.tile([MID, 4, W], bf16, name="xr")
                    nc.scalar.activation(xr[:], acc[:], Act.Relu,
                                         bias=bmid[:], scale=1.0)
                    accg = psG.tile([KD, 4, W], f32, name="accg")
                    for half in (0, 1):
                        nc.tensor.matmul(out=accg[:, 2 * half:2 * half + 2, :],
                                         lhsT=w2[:],
                                         rhs=xr[:, 2 * half:2 * half + 2, :],
                                         start=True, stop=True)
                    es = esp.tile([KD, 4, W], bf16, name="es")
                    nc.scalar.activation(es[:], accg[:], Act.Exp,
                                         bias=b2c[:], scale=1.0)
                    # in-phase softmax: PE reduces the 4 directions per k
                    # (ksum block matrix, f32 psum), DVE fast-reciprocal +
                    # in-place normalize -- all hidden under the conv
                    sps = psN.tile([KD, 4, W], f32, name="sps")
                    for half in (0, 1):
                        nc.tensor.matmul(out=sps[:, 2 * half:2 * half + 2, :],
                                         lhsT=ksum[:],
                                         rhs=es[:, 2 * half:2 * half + 2, :],
                                         start=True, stop=True)
                    rq = rqp.tile([KD, 4, W], f32, name="rq")
                    nc.vector.reciprocal_approx_fast(out=rq[:], in_=sps[:])
                    nc.vector.tensor_tensor(out=es[:], in0=es[:], in1=rq[:],
                                            op=Alu.mult)
                    # es channels are d-major (d*K+k) so the scatter only
                    # permutes free dims on the dest side (partition-dim
                    # splits in DMA APs mis-lower to slot-crossing strides)
                    for r in range(4):
                        eng = nc.sync if r % 2 == 0 else nc.scalar
                        eng.dma_start(
                            out=e_all[y + r:y + r + 1, :, :, :].rearrange(
                                "p d k w -> p (d k) w"),
                            in_=es[:, r, :])

                for g in range(n_groups):
                    ft = frp.tile([128, 2, RG, WP], bf16, name=f"ft{g}", tag="ft")
                    # 4-row slices so each group load spreads across 8 DMA
                    # engines (a single big DMA serializes on one engine and
                    # stalls the conv at group boundaries)
                    for c in range(2):
                        for q in range(0, RG, 4):
                            nc.sync.dma_start(
                                out=ft[:, c, q:q + 4, :],
                                in_=feats_d[c * 128:(c + 1) * 128,
                                            g * RG + q:g * RG + q + 4, :])
                    ftiles[g] = ft
                    if g == 0:
                        quads = [0, 4, 8]
                    elif g == n_groups - 1:
                        quads = [g * RG - 4, g * RG, g * RG + 4, g * RG + 8,
                                 g * RG + 12]
                    else:
                        quads = [g * RG - 4, g * RG, g * RG + 4, g * RG + 8]
                    for y in quads:
                        emit_quad(y)

            # ============ gate pre-shift ============
            with tc.tile_pool(name="work", bufs=1) as wp, \
                 tc.tile_pool(name="psS", bufs=4, space="PSUM") as psS:
                u_w = wp.tile([128, KW], bf16)
                d_w = wp.tile([128, KW], bf16)
                e1b = wp.tile([128, KW], bf16)
                e2b = wp.tile([128, KW], bf16)

                ev = [e_all[:, d, :, :] for d in range(4)]
                uv = u_w[:].rearrange("p (k w) -> p k w", k=K)
                dv = d_w[:].rearrange("p (k w) -> p k w", k=K)

                # pre-shift: gup = S_dn @ gu ; gdp = S_up @ gd (normalized)
                for dsel, dst, mat in ((2, gup, s_dn), (3, gdp, s_up)):
                    for (o, n) in CHUNKS:
                        ps = psS.tile([128, 1024], f32, name="shps", tag="ps")
                        for so in range(0, n, 512):
                            sn = min(512, n - so)
                            k0 = (o + so) // W
                            nc.tensor.matmul(
                                out=ps[:, so:so + sn], lhsT=mat[:],
                                rhs=e_all[:, dsel, k0:k0 + sn // W, :],
                                start=True, stop=True)
                        nc.scalar.activation(dst[:, o:o + n], ps[:, 0:n],
                                             Act.Copy, scale=1.0)

                if debug:
                    nc.sync.dma_start(
                        out=d_eall[:],
                        in_=e_all[:].rearrange("p d k w -> p (d k w)"))
                    nc.sync.dma_start(out=d_gup[:], in_=gup[:])
                    nc.sync.dma_start(out=d_gdp[:], in_=gdp[:])

                # ================= phase B: recurrence =================
                # nxt = S_up@(gup.h) + S_dn@(gdp.h) + I@(gl.left) + I@(gr.right)
                # all accumulated in PSUM chunks; ACT drains straight to nxt
                # (avoids slow f32->bf16 TT down-converts on DVE/Pool)
                ev1 = e1b[:].rearrange("p (k w) -> p k w", k=K)
                ev2 = e2b[:].rearrange("p (k w) -> p k w", k=K)
                gupv = gup[:].rearrange("p (k w) -> p k w", k=K)
                gdpv = gdp[:].rearrange("p (k w) -> p k w", k=K)
                cur, nxt = h_a, h_b
                for t in range(T_STEPS):
                    # all four gate-mults on DVE (all-bf16 2x; Pool's software
                    # TT is ~3.5x slower and adds SBUF contention); sliced
                    # per psum chunk so they pipeline with drains and matmuls
                    curv = cur[:, :, 1:1 + W]
                    for (o, n) in CHUNKS:
                        k0, nk = o // W, n // W
                        nc.vector.tensor_tensor(
                            out=uv[:, k0:k0 + nk], in0=gupv[:, k0:k0 + nk],
                            in1=curv[:, k0:k0 + nk], op=Alu.mult)
                        nc.vector.tensor_tensor(
                            out=dv[:, k0:k0 + nk], in0=gdpv[:, k0:k0 + nk],
                            in1=curv[:, k0:k0 + nk], op=Alu.mult)
                        nc.vector.tensor_tensor(
                            out=ev1[:, k0:k0 + nk], in0=ev[0][:, k0:k0 + nk],
                            in1=cur[:, k0:k0 + nk, 0:W], op=Alu.mult)
                        nc.vector.tensor_tensor(
                            out=ev2[:, k0:k0 + nk], in0=ev[1][:, k0:k0 + nk],
                            in1=cur[:, k0:k0 + nk, 2:2 + W], op=Alu.mult)
                    for wave in (CHUNKS[0:4], CHUNKS[4:5]):
                        pss = [psS.tile([128, 1024], f32, name="bps", tag="ps")
                               for _ in wave]
                        for wi, (wt, src) in enumerate(
                                ((s_up, u_w), (s_dn, d_w),
                                 (ident, e1b), (ident, e2b))):
                            for (o, n), ps in zip(wave, pss):
                                for so in range(0, n, 512):
                                    sn = min(512, n - so)
                                    nc.tensor.matmul(
                                        out=ps[:, so:so + sn], lhsT=wt[:],
                                        rhs=src[:, o + so:o + so + sn],
                                        start=(wi == 0), stop=(wi == 3))
                        for (o, n), ps in zip(wave, pss):
                            k0 = o // W
                            nk = n // W
                            nc.scalar.activation(
                                nxt[:, k0:k0 + nk, 1:1 + W],
                                ps[:, 0:n].rearrange("p (k w) -> p k w", k=nk),
                                Act.Copy, scale=1.0)
                    if debug and t == 0:
                        nc.sync.dma_start(
                            out=d_h1[:],
                            in_=nxt[:].rearrange("p k w -> p (k w)"))
                        nc.sync.dma_start(out=d_ps[:], in_=u_w[:])
                    cur, nxt = nxt, cur

                with tc.tile_pool(name="outp", bufs=1) as op_:
                    of32 = op_.tile([128, KW], f32)
                    nc.scalar.activation(
                        of32[:].rearrange("p (k w) -> p k w", k=K),
                        cur[:, :, 1:1 + W], Act.Copy, scale=1.0)
                    nc.sync.dma_start(out=out_d[:], in_=of32[:])

    nc.compile()
    return nc


_NC_CACHE = None


def kernel(feats, logits, w1, gamma, beta, mean, var, w2, b2):
    global _NC_CACHE
    from concourse.bass_utils import run_bass_kernel_spmd

    feats = np.asarray(feats, dtype=np.float32)
    logits = np.asarray(logits, dtype=np.float32)
    w1 = np.asarray(w1, dtype=np.float32)
    w2 = np.asarray(w2, dtype=np.float32)
    b2 = np.asarray(b2, dtype=np.float32)
    gamma = np.asarray(gamma, dtype=np.float32)
    beta = np.asarray(beta, dtype=np.float32)
    mean = np.asarray(mean, dtype=np.float32)
    var = np.asarray(var, dtype=np.float32)

    inv = gamma / np.sqrt(var + EPS)
    w1f = w1 * inv[:, None, None, None]               # [MID,CIN,3,3]
    bmid = (beta - mean * inv).astype(np.float32)[:, None]
    # [cin_in_chunk 128, chunk 2, tap 9, mid 128]
    w1t = np.ascontiguousarray(
        w1f.transpose(1, 2, 3, 0)                     # [CIN,3,3,MID]
        .reshape(2, 128, 9, MID)
        .transpose(1, 0, 2, 3)).astype(BF)
    # channel order d-major (c' = d*K + k) to keep the gate scatter free of
    # partition-dim splits
    w2dm = w2.reshape(K, 4, MID).transpose(1, 0, 2).reshape(KD, MID)
    w2t = np.ascontiguousarray(w2dm.T).astype(BF)
    b2c = np.ascontiguousarray(
        b2.reshape(K, 4).T.reshape(KD)[:, None]).astype(np.float32)
    s_up = np.eye(128, k=1, dtype=np.float32).astype(BF)   # out[m]=in[m-1]
    s_dn = np.eye(128, k=-1, dtype=np.float32).astype(BF)  # out[m]=in[m+1]
    ident = np.eye(128, dtype=np.float32).astype(BF)
    # ksum[c', c] = 1 iff same k (channels d-major: k = c % 19)
    cc = np.arange(KD)
    ksum = (cc[:, None] % K == cc[None, :] % K).astype(np.float32).astype(BF)

    feats_p = np.zeros((B, CIN, H, WP), dtype=BF)
    feats_p[:, :, :, 1:1 + W] = feats.astype(BF)
    logits_p = np.zeros((B, K, H, WP), dtype=BF)
    logits_p[:, :, :, 1:1 + W] = logits.astype(BF)

    debug = bool(os.environ.get("KDEBUG"))
    if _NC_CACHE is None:
        _NC_CACHE = _build(debug=debug)
    nc = _NC_CACHE

    in_maps = []
    for i in range(B):
        in_maps.append({
            "feats": feats_p[i],
            "logits": logits_p[i],
            "w1t": w1t, "bmid": bmid, "w2t": w2t, "b2": b2c,
            "sup": s_up, "sdn": s_dn, "ident": ident, "ksum": ksum,
        })

    trace = bool(os.environ.get("KTRACE"))
    res = run_bass_kernel_spmd(nc, in_maps, list(range(B)), trace=trace)
    if trace and res.exec_time_ns is not None:
        print(f"HW exec time: {res.exec_time_ns} ns")
    if debug:
        kernel.dbg = {k: np.asarray(res.results[0][k], dtype=np.float32)
                      for k in ("d_eall", "d_gup", "d_gdp", "d_h1", "d_ps")}
    out = np.stack([res.results[i]["out"] for i in range(B)], axis=0)
    # [B, H, K*W] -> [B, K, H, W]
    out = out.reshape(B, H, K, W).transpose(0, 2, 1, 3)
    return np.ascontiguousarray(out.astype(np.float32))


if __name__ == "__main__":
    rng = np.random.default_rng(0)
    ins = {
        "feats": rng.standard_normal((B, CIN, H, W), dtype=np.float32),
        "logits": rng.standard_normal((B, K, H, W), dtype=np.float32),
        "w1": rng.standard_normal((MID, CIN, 3, 3), dtype=np.float32) / 48.0,
        "gamma": rng.standard_normal(MID).astype(np.float32) * 0.1 + 1.0,
        "beta": rng.standard_normal(MID).astype(np.float32) * 0.1,
        "mean": rng.standard_normal(MID).astype(np.float32) * 0.1,
        "var": rng.random(MID).astype(np.float32) + 0.5,
        "w2": rng.standard_normal((KD, MID, 1, 1)).astype(np.float32) / 11.3,
        "b2": rng.standard_normal(KD).astype(np.float32) * 0.01,
    }
    o = kernel(**ins)
    print("kernel out", o.shape, o.dtype, np.abs(o).mean())


# revision 46
# speedup vs baseline: 2.0655x; 1.0240x over previous
"""MCSPN Trainium2 kernel v2: bf16 guidance convs + softmax gates + 4-step CSPN.

Data-parallel over batch: 8 images -> 8 NeuronCores, one image per core.
Host pre-pads feats/logits (x-guard columns) and casts to bf16 so every
DMA moves large contiguous packets and DVE runs in 2x mode.

Per core:
  phase A: conv3x3 over row-QUADS (N=1024 bf16 matmuls, 18 accum MMs/quad)
           -> bias+ReLU (ACT, bf16 out) -> conv1x1 (bf16) -> exp (ACT)
           -> per-row DMA scatter into gate layout e_all [128 x 19*4*256]
  softmax: 3 adds + reciprocal + 4 muls on [128, 19, 256] bf16 views (DVE 2x)
  gate pre-shift: gup[y]=gu[y+1], gdp[y]=gd[y-1] via PE shift-matmuls (once)
  phase B: 4 steps; per step: 4 gate-muls + 1 add (bf16 2x, DVE/Pool),
           up+down shift-matmuls chain-accumulated into shared PSUM chunks,
           10 chunk adds write next-h directly.
"""
import os
import sys

sys.path.insert(0, "/opt/trn_rl_repo")

import numpy as np
import ml_dtypes

B, CIN, H, W = 8, 256, 128, 256
K = 19
MID = 128
KD = 4 * K  # 76
EPS = 1e-5
T_STEPS = 4
WP = W + 2   # guarded row width (258)
RG = 16      # feats rows per DMA group
KW = K * W   # 4864 packed gate/h width
BF = ml_dtypes.bfloat16

# psum chunking of the packed [128, K*W] plane: 4x1024 + 1x768
# (each chunk is one 4KB psum slot = 2 banks; k-aligned since 1024 = 4*W)
CHUNKS = [(j * 1024, 1024) for j in range(4)] + [(4096, 768)]


def _build(debug=False):
    import concourse.bacc as bacc
    import concourse.mybir as mybir
    import concourse.tile as tile

    f32 = mybir.dt.float32
    bf16 = mybir.dt.bfloat16
    Act = mybir.ActivationFunctionType
    Alu = mybir.AluOpType

    nc = bacc.Bacc("TRN2", target_bir_lowering=False)

    feats_d = nc.dram_tensor("feats", [CIN, H, WP], bf16, kind="ExternalInput")
    logits_d = nc.dram_tensor("logits", [K, H, WP], bf16, kind="ExternalInput")
    w1t_d = nc.dram_tensor("w1t", [128, 2, 9, MID], bf16, kind="ExternalInput")
    bmid_d = nc.dram_tensor("bmid", [MID, 1], f32, kind="ExternalInput")
    w2t_d = nc.dram_tensor("w2t", [MID, KD], bf16, kind="ExternalInput")
    b2_d = nc.dram_tensor("b2", [KD, 1], f32, kind="ExternalInput")
    sup_d = nc.dram_tensor("sup", [128, 128], bf16, kind="ExternalInput")
    sdn_d = nc.dram_tensor("sdn", [128, 128], bf16, kind="ExternalInput")
    ident_d = nc.dram_tensor("ident", [128, 128], bf16, kind="ExternalInput")
    ksum_d = nc.dram_tensor("ksum", [KD, KD], bf16, kind="ExternalInput")
    out_d = nc.dram_tensor("out", [H, KW], bf16, kind="ExternalOutput")
    if debug:
        d_eall = nc.dram_tensor("d_eall", [128, K * 4 * W], bf16,
                                kind="ExternalOutput")
        d_gup = nc.dram_tensor("d_gup", [128, KW], bf16, kind="ExternalOutput")
        d_gdp = nc.dram_tensor("d_gdp", [128, KW], bf16, kind="ExternalOutput")
        d_h1 = nc.dram_tensor("d_h1", [128, K * WP], bf16,
                              kind="ExternalOutput")
        d_ps = nc.dram_tensor("d_ps", [128, KW], bf16, kind="ExternalOutput")

    with nc.allow_low_precision(reason="bf16 kernel; rel-err gate is 2e-2"), \
         tile.TileContext(nc) as tc:
        with tc.tile_pool(name="persist", bufs=1) as pp:
            e_all = pp.tile([128, 4, K, W], bf16)      # gates, d-MAJOR
            h_a = pp.tile([128, K, WP], bf16)
            h_b = pp.tile([128, K, WP], bf16)
            gup = pp.tile([128, KW], bf16)             # gu shifted: gup[y]=gu[y+1]
            gdp = pp.tile([128, KW], bf16)             # gd shifted: gdp[y]=gd[y-1]
            s_up = pp.tile([128, 128], bf16)
            s_dn = pp.tile([128, 128], bf16)
            ident = pp.tile([128, 128], bf16)
            ksum = pp.tile([KD, KD], bf16)
            w2 = pp.tile([MID, KD], bf16)
            bmid = pp.tile([MID, 1], f32)
            b2c = pp.tile([KD, 1], f32)

            nc.sync.dma_start(out=s_up[:], in_=sup_d[:])
            nc.sync.dma_start(out=s_dn[:], in_=sdn_d[:])
            nc.sync.dma_start(out=ident[:], in_=ident_d[:])
            nc.sync.dma_start(out=ksum[:], in_=ksum_d[:])
            nc.sync.dma_start(out=w2[:], in_=w2t_d[:])
            nc.sync.dma_start(out=bmid[:], in_=bmid_d[:])
            nc.sync.dma_start(out=b2c[:], in_=b2_d[:])

            # h_b guard columns zero (never written by steps)
            nc.vector.memset(h_b[:, :, 0:WP:WP - 1], 0.0)

            # ================= phase A: guidance =================
            with tc.tile_pool(name="w1p", bufs=1) as w1p, \
                 tc.tile_pool(name="frows", bufs=4) as frp, \
                 tc.tile_pool(name="xrow", bufs=3) as xrp, \
                 tc.tile_pool(name="estrip", bufs=3) as esp, \
                 tc.tile_pool(name="rqp", bufs=2) as rqp, \
                 tc.tile_pool(name="psA", bufs=2, space="PSUM") as psA, \
                 tc.tile_pool(name="psG", bufs=1, space="PSUM") as psG, \
                 tc.tile_pool(name="psN", bufs=1, space="PSUM") as psN:
                w1 = w1p.tile([128, 2, 9, MID], bf16)
                # split across engines so the first conv matmul isn't gated
                # on one long single-engine weight load
                for c in range(2):
                    for tp in range(0, 9, 3):
                        nc.sync.dma_start(out=w1[:, c, tp:tp + 3, :],
                                          in_=w1t_d[:, c, tp:tp + 3, :])

                n_groups = H // RG
                ftiles = [None] * n_groups

                def emit_quad(y):
                    # PSUM bank limit: each matmul out <= 512 f32, so the
                    # 4-row quad accumulates as two independent 2-row halves
                    acc = psA.tile([MID, 4, W], f32, name="acc")
                    for half in (0, 1):
                        y2 = y + 2 * half
                        mms = []  # (c, tap, rhs_ap, out_ap)
                        for ky in (1, 0, 2):
                            iy0 = y2 + ky - 1
                            a0 = max(0, -iy0)
                            a1 = min(2, H - iy0)
                            runs = []
                            s = iy0 + a0
                            while s < iy0 + a1:
                                e = min(iy0 + a1, (s // RG + 1) * RG)
                                runs.append((s, e))
                                s = e
                            for c in range(2):
                                for kx in range(3):
                                    for (s, e) in runs:
                                        g = s // RG
                                        rhs = ftiles[g][:, c,
                                                        s % RG:s % RG + (e - s),
                                                        kx:kx + W]
                                        oap = acc[:, 2 * half + (s - iy0):
                                                   2 * half + (e - iy0), :]
                                        mms.append((c, ky * 3 + kx, rhs, oap))
                        for i, (c, tap, rhs, oap) in enumerate(mms):
                            nc.tensor.matmul(out=oap, lhsT=w1[:, c, tap, :],
                                             rhs=rhs, start=(i == 0),
                                             stop=(i == len(mms) - 1))
                    xr = xrp.tile([MID, 4, W], bf16, name="xr")
                    nc.scalar.activation(xr[:], acc[:], Act.Relu,
                                         bias=bmid[:], scale=1.0)
                    accg = psG.tile([KD, 4, W], f32, name="accg")
                    for half in (0, 1):
                        nc.tensor.matmul(out=accg[:, 2 * half:2 * half + 2, :],
                                         lhsT=w2[:],
                                         rhs=xr[:, 2 * half:2 * half + 2, :],
                                         start=True, stop=True)
                    es = esp.tile([KD, 4, W], bf16, name="es")
                    nc.scalar.activation(es[:], accg[:], Act.Exp,
                                         bias=b2c[:], scale=1.0)
                    # in-phase softmax: PE reduces the 4 directions per k
                    # (ksum block matrix, f32 psum), DVE fast-reciprocal +
                    # in-place normalize -- all hidden under the conv
                    sps = psN.tile([KD, 4, W], f32, name="sps")
                    for half in (0, 1):
                        nc.tensor.matmul(out=sps[:, 2 * half:2 * half + 2, :],
                                         lhsT=ksum[:],
                                         rhs=es[:, 2 * half:2 * half + 2, :],
                                         start=True, stop=True)
                    rq = rqp.tile([KD, 4, W], f32, name="rq")
                    nc.vector.reciprocal_approx_fast(out=rq[:], in_=sps[:])
                    nc.vector.tensor_tensor(out=es[:], in0=es[:], in1=rq[:],
                                            op=Alu.mult)
                    # es channels are d-major (d*K+k) so the scatter only
                    # permutes free dims on the dest side (partition-dim
                    # splits in DMA APs mis-lower to slot-crossing strides)
                    for r in range(4):
                        eng = nc.sync if r % 2 == 0 else nc.scalar
                        eng.dma_start(
                            out=e_all[y + r:y + r + 1, :, :, :].rearrange(
                                "p d k w -> p (d k) w"),
                            in_=es[:, r, :])

                for g in range(n_groups):
                    ft = frp.tile([128, 2, RG, WP], bf16, name=f"ft{g}", tag="ft")
                    # 4-row slices so each group load spreads across 8 DMA
                    # engines (a single big DMA serializes on one engine and
                    # stalls the conv at group boundaries)
                    for c in range(2):
                        for q in range(0, RG, 4):
                            nc.sync.dma_start(
                                out=ft[:, c, q:q + 4, :],
                                in_=feats_d[c * 128:(c + 1) * 128,
                                            g * RG + q:g * RG + q + 4, :])
                    ftiles[g] = ft
                    if g == 0:
                        quads = [0, 4, 8]
                    elif g == n_groups - 1:
                        quads = [g * RG - 4, g * RG, g * RG + 4, g * RG + 8,
                                 g * RG + 12]
                    else:
                        quads = [g * RG - 4, g * RG, g * RG + 4, g * RG + 8]
                    for y in quads:
                        emit_quad(y)

            # ============ gate pre-shift ============
            with tc.tile_pool(name="work", bufs=1) as wp, \
                 tc.tile_pool(name="psS", bufs=4, space="PSUM") as psS:
                u_w = wp.tile([128, KW], bf16)
                d_w = wp.tile([128, KW], bf16)
                e1b = wp.tile([128, KW], bf16)
                e2b = wp.tile([128, KW], bf16)

                ev = [e_all[:, d, :, :] for d in range(4)]
                uv = u_w[:].rearrange("p (k w) -> p k w", k=K)
                dv = d_w[:].rearrange("p (k w) -> p k w", k=K)

                # h0 = logits (host-padded guards already zero); loaded here
                # so these DMAs don't delay the phase A feats stream
                for k in range(K):
                    eng = nc.sync if k % 2 == 0 else nc.scalar
                    eng.dma_start(out=h_a[:, k, :], in_=logits_d[k])

                # pre-shift: gup = S_dn @ gu ; gdp = S_up @ gd (normalized)
                for dsel, dst, mat in ((2, gup, s_dn), (3, gdp, s_up)):
                    for (o, n) in CHUNKS:
                        ps = psS.tile([128, 1024], f32, name="shps", tag="ps")
                        for so in range(0, n, 512):
                            sn = min(512, n - so)
                            k0 = (o + so) // W
                            nc.tensor.matmul(
                                out=ps[:, so:so + sn], lhsT=mat[:],
                                rhs=e_all[:, dsel, k0:k0 + sn // W, :],
                                start=True, stop=True)
                        nc.scalar.activation(dst[:, o:o + n], ps[:, 0:n],
                                             Act.Copy, scale=1.0)

                if debug:
                    nc.sync.dma_start(
                        out=d_eall[:],
                        in_=e_all[:].rearrange("p d k w -> p (d k w)"))
                    nc.sync.dma_start(out=d_gup[:], in_=gup[:])
                    nc.sync.dma_start(out=d_gdp[:], in_=gdp[:])

                # ================= phase B: recurrence =================
                # nxt = S_up@(gup.h) + S_dn@(gdp.h) + I@(gl.left) + I@(gr.right)
                # all accumulated in PSUM chunks; ACT drains straight to nxt
                # (avoids slow f32->bf16 TT down-converts on DVE/Pool)
                ev1 = e1b[:].rearrange("p (k w) -> p k w", k=K)
                ev2 = e2b[:].rearrange("p (k w) -> p k w", k=K)
                gupv = gup[:].rearrange("p (k w) -> p k w", k=K)
                gdpv = gdp[:].rearrange("p (k w) -> p k w", k=K)
                cur, nxt = h_a, h_b
                for t in range(T_STEPS):
                    # all four gate-mults on DVE (all-bf16 2x; Pool's software
                    # TT is ~3.5x slower and adds SBUF contention); sliced
                    # per psum chunk so they pipeline with drains and matmuls
                    curv = cur[:, :, 1:1 + W]
                    for (o, n) in CHUNKS:
                        k0, nk = o // W, n // W
                        nc.vector.tensor_tensor(
                            out=uv[:, k0:k0 + nk], in0=gupv[:, k0:k0 + nk],
                            in1=curv[:, k0:k0 + nk], op=Alu.mult)
                        nc.vector.tensor_tensor(
                            out=dv[:, k0:k0 + nk], in0=gdpv[:, k0:k0 + nk],
                            in1=curv[:, k0:k0 + nk], op=Alu.mult)
                        nc.vector.tensor_tensor(
                            out=ev1[:, k0:k0 + nk], in0=ev[0][:, k0:k0 + nk],
                            in1=cur[:, k0:k0 + nk, 0:W], op=Alu.mult)
                        nc.vector.tensor_tensor(
                            out=ev2[:, k0:k0 + nk], in0=ev[1][:, k0:k0 + nk],
                            in1=cur[:, k0:k0 + nk, 2:2 + W], op=Alu.mult)
                    for wave in (CHUNKS[0:4], CHUNKS[4:5]):
                        pss = [psS.tile([128, 1024], f32, name="bps", tag="ps")
                               for _ in wave]
                        for wi, (wt, src) in enumerate(
                                ((s_up, u_w), (s_dn, d_w),
                                 (ident, e1b), (ident, e2b))):
                            for (o, n), ps in zip(wave, pss):
                                for so in range(0, n, 512):
                                    sn = min(512, n - so)
                                    nc.tensor.matmul(
                                        out=ps[:, so:so + sn], lhsT=wt[:],
                                        rhs=src[:, o + so:o + so + sn],
                                        start=(wi == 0), stop=(wi == 3))
                        for (o, n), ps in zip(wave, pss):
                            k0 = o // W
                            nk = n // W
                            nc.scalar.activation(
                                nxt[:, k0:k0 + nk, 1:1 + W],
                                ps[:, 0:n].rearrange("p (k w) -> p k w", k=nk),
                                Act.Copy, scale=1.0)
                    if debug and t == 0:
                        nc.sync.dma_start(
                            out=d_h1[:],
                            in_=nxt[:].rearrange("p k w -> p (k w)"))
                        nc.sync.dma_start(out=d_ps[:], in_=u_w[:])
                    cur, nxt = nxt, cur

                with tc.tile_pool(name="outp", bufs=1) as op_:
                    # pack interior to contiguous bf16 (exact: h is bf16);
                    # host widens to f32
                    obf = op_.tile([128, KW], bf16)
                    nc.scalar.activation(
                        obf[:].rearrange("p (k w) -> p k w", k=K),
                        cur[:, :, 1:1 + W], Act.Copy, scale=1.0)
                    nc.sync.dma_start(out=out_d[:], in_=obf[:])

    nc.compile()
    return nc


_NC_CACHE = None


def kernel(feats, logits, w1, gamma, beta, mean, var, w2, b2):
    global _NC_CACHE
    from concourse.bass_utils import run_bass_kernel_spmd

    feats = np.asarray(feats, dtype=np.float32)
    logits = np.asarray(logits, dtype=np.float32)
    w1 = np.asarray(w1, dtype=np.float32)
    w2 = np.asarray(w2, dtype=np.float32)
    b2 = np.asarray(b2, dtype=np.float32)
    gamma = np.asarray(gamma, dtype=np.float32)
    beta = np.asarray(beta, dtype=np.float32)
    mean = np.asarray(mean, dtype=np.float32)
    var = np.asarray(var, dtype=np.float32)

    inv = gamma / np.sqrt(var + EPS)
    w1f = w1 * inv[:, None, None, None]               # [MID,CIN,3,3]
    bmid = (beta - mean * inv).astype(np.float32)[:, None]
    # [cin_in_chunk 128, chunk 2, tap 9, mid 128]
    w1t = np.ascontiguousarray(
        w1f.transpose(1, 2, 3, 0)                     # [CIN,3,3,MID]
        .reshape(2, 128, 9, MID)
        .transpose(1, 0, 2, 3)).astype(BF)
    # channel order d-major (c' = d*K + k) to keep the gate scatter free of
    # partition-dim splits
    w2dm = w2.reshape(K, 4, MID).transpose(1, 0, 2).reshape(KD, MID)
    w2t = np.ascontiguousarray(w2dm.T).astype(BF)
    b2c = np.ascontiguousarray(
        b2.reshape(K, 4).T.reshape(KD)[:, None]).astype(np.float32)
    s_up = np.eye(128, k=1, dtype=np.float32).astype(BF)   # out[m]=in[m-1]
    s_dn = np.eye(128, k=-1, dtype=np.float32).astype(BF)  # out[m]=in[m+1]
    ident = np.eye(128, dtype=np.float32).astype(BF)
    # ksum[c', c] = 1 iff same k (channels d-major: k = c % 19)
    cc = np.arange(KD)
    ksum = (cc[:, None] % K == cc[None, :] % K).astype(np.float32).astype(BF)

    feats_p = np.zeros((B, CIN, H, WP), dtype=BF)
    feats_p[:, :, :, 1:1 + W] = feats.astype(BF)
    logits_p = np.zeros((B, K, H, WP), dtype=BF)
    logits_p[:, :, :, 1:1 + W] = logits.astype(BF)

    debug = bool(os.environ.get("KDEBUG"))
    if _NC_CACHE is None:
        _NC_CACHE = _build(debug=debug)
    nc = _NC_CACHE

    in_maps = []
    for i in range(B):
        in_maps.append({
            "feats": feats_p[i],
            "logits": logits_p[i],
            "w1t": w1t, "bmid": bmid, "w2t": w2t, "b2": b2c,
            "sup": s_up, "sdn": s_dn, "ident": ident, "ksum": ksum,
        })

    trace = bool(os.environ.get("KTRACE"))
    res = run_bass_kernel_spmd(nc, in_maps, list(range(B)), trace=trace)
    if trace and res.exec_time_ns is not None:
        print(f"HW exec time: {res.exec_time_ns} ns")
    if debug:
        kernel.dbg = {k: np.asarray(res.results[0][k], dtype=np.float32)
                      for k in ("d_eall", "d_gup", "d_gdp", "d_h1", "d_ps")}
    out = np.stack([res.results[i]["out"] for i in range(B)], axis=0)
    # [B, H, K*W] -> [B, K, H, W]
    out = out.reshape(B, H, K, W).transpose(0, 2, 1, 3)
    return np.ascontiguousarray(out.astype(np.float32))


if __name__ == "__main__":
    rng = np.random.default_rng(0)
    ins = {
        "feats": rng.standard_normal((B, CIN, H, W), dtype=np.float32),
        "logits": rng.standard_normal((B, K, H, W), dtype=np.float32),
        "w1": rng.standard_normal((MID, CIN, 3, 3), dtype=np.float32) / 48.0,
        "gamma": rng.standard_normal(MID).astype(np.float32) * 0.1 + 1.0,
        "beta": rng.standard_normal(MID).astype(np.float32) * 0.1,
        "mean": rng.standard_normal(MID).astype(np.float32) * 0.1,
        "var": rng.random(MID).astype(np.float32) + 0.5,
        "w2": rng.standard_normal((KD, MID, 1, 1)).astype(np.float32) / 11.3,
        "b2": rng.standard_normal(KD).astype(np.float32) * 0.01,
    }
    o = kernel(**ins)
    print("kernel out", o.shape, o.dtype, np.abs(o).mean())
